# revision 1
# baseline (speedup 1.0000x reference)
"""Distributed Bass kernel for nn_Attention (B=2, N=2048, D=1024, H=16, DH=64) on 8 trn2 cores.

Sharding: data-parallel over batch (cores 0-3 -> b=0, 4-7 -> b=1), tensor-parallel
over heads (4 heads / 256 inner features per core).  Per core:
  q/k/v projections of x[b] against its 256-wide weight slices (f32r matmuls),
  RMSNorm via partial sum-of-squares + tiny AllReduce over the 4-core batch group,
  RoPE via host cos/sin tables + pair-swap permutation matmul,
  attention (scores transposed [m,n], softmax denominator via ones-row in v),
  8-core AllGather of attention outputs (bf16) + indirect-DMA slice pick,
  output projection of this core's 512-row n-slice (bf16) + bias.
Host assembles the (2, 2048, 1024) output from the 8 (512, 1024) shards.
"""
import os
import sys

for _p in ("/opt/trn_rl_repo", "/root/.axon_site/_ro/trn_rl_repo"):
    if os.path.isdir(_p) and _p not in sys.path:
        sys.path.insert(0, _p)

import numpy as np
import concourse.bass as bass
import concourse.mybir as mybir
import concourse.tile as tile
from concourse import bacc
from concourse.bass_utils import run_bass_kernel_spmd

dt = mybir.dt
AF = mybir.ActivationFunctionType
F32, F32R, BF16, I32 = dt.float32, dt.float32r, dt.bfloat16, dt.int32

B, N, D = 2, 2048, 1024
H, DH = 16, 64
HPC = 4            # heads per core
FPC = HPC * DH     # 256 inner features per core
KC = D // 128      # 8 contraction chunks
FC = FPC // 128    # 2 feature chunks per core
NJ = N // 512      # 4 n-chunks
NT = N // 128      # 16 n-tiles / m-tiles
EPS = 1e-6
CORES = 8
GROUPS4 = [[0, 1, 2, 3], [4, 5, 6, 7]]
GROUP8 = [list(range(CORES))]

_CACHED_NC = None


def _const_tile(nc, pool, shape, val, dtype, tag):
    t32 = pool.tile(shape, F32, tag=tag + "_f32", name=tag + "_f32")
    nc.gpsimd.memset(t32[:], val)
    if dtype == F32:
        return t32, t32
    t = pool.tile(shape, dtype, tag=tag, name=tag)
    nc.vector.tensor_copy(t[:], t32[:])
    return t, t32


def build(dbg=False):
    nc = bacc.Bacc("TRN2", target_bir_lowering=False, debug=False, num_devices=CORES)

    xT = nc.dram_tensor("xT", [D, N], F32, kind="ExternalInput")
    wq_d = nc.dram_tensor("wq", [D, FPC], F32, kind="ExternalInput")
    wk_d = nc.dram_tensor("wk", [D, FPC], F32, kind="ExternalInput")
    wv_d = nc.dram_tensor("wv", [D, FPC], F32, kind="ExternalInput")
    wo_d = nc.dram_tensor("wo", [D, D], F32, kind="ExternalInput")
    bo_d = nc.dram_tensor("bo", [1, D], F32, kind="ExternalInput")
    wqc_d = nc.dram_tensor("wqc", [128, FC], F32, kind="ExternalInput")
    wkc_d = nc.dram_tensor("wkc", [128, FC], F32, kind="ExternalInput")
    cos_d = nc.dram_tensor("cos_t", [128, N], F32, kind="ExternalInput")
    sin_d = nc.dram_tensor("sin_t", [128, N], F32, kind="ExternalInput")
    gidx_d = nc.dram_tensor("gidx", [128, KC], I32, kind="ExternalInput")
    out_d = nc.dram_tensor("out", [512, D], F32, kind="ExternalOutput")
    if dbg:
        dbg_qn = nc.dram_tensor("dbg_qn", [128, FC, N], F32, kind="ExternalOutput")
        dbg_kt = nc.dram_tensor("dbg_kt", [128, FC, N], F32, kind="ExternalOutput")
        dbg_va = nc.dram_tensor(
            "dbg_va", [128, NT, HPC, DH + 1], BF16, kind="ExternalOutput"
        )
        dbg_osb = nc.dram_tensor("dbg_osb", [128, FC, N], BF16, kind="ExternalOutput")
        dbg_og = nc.dram_tensor("dbg_og", [128, KC, 512], BF16, kind="ExternalOutput")

    # collective bounce buffers
    ssq_in = nc.dram_tensor("ssq_in", [2, N], F32)
    ssq_out = nc.dram_tensor("ssq_out", [2, N], F32)
    ag_in = [nc.dram_tensor(f"ag_in{c}", [128, N], BF16) for c in range(FC)]
    ag_out = [
        nc.dram_tensor(f"ag_out{c}", [CORES * 128, N], BF16, addr_space="Shared")
        for c in range(FC)
    ]

    with tile.TileContext(nc) as tc:
        with tc.tile_pool(name="persist", bufs=1) as pp:
            # ---- whole-kernel constants and tensors -----------------------
            ones_col_r, ones_col32 = _const_tile(nc, pp, [128, 1], 1.0, F32R, "onesc")
            ones_row_r, ones_row32 = _const_tile(nc, pp, [1, 128], 1.0, F32R, "onesr")
            ones_row_bf = pp.tile([1, 128], BF16, tag="onesrbf")
            nc.vector.tensor_copy(ones_row_bf[:], ones_row32[:])
            one1_r, one1_f32 = _const_tile(nc, pp, [33, 1], 1.0, F32R, "one1")
            # eps bias values aligned with their consumers' base partitions
            eps_both = pp.tile([64, 1], F32, tag="eps")
            nc.gpsimd.memset(eps_both[0:1, :], EPS)
            nc.gpsimd.memset(eps_both[32:33, :], 64.0 * EPS)

            wqc_sb = pp.tile([128, FC], F32, tag="wqc")
            wkc_sb = pp.tile([128, FC], F32, tag="wkc")
            nc.sync.dma_start(out=wqc_sb[:], in_=wqc_d[:])
            nc.sync.dma_start(out=wkc_sb[:], in_=wkc_d[:])
            gidx_sb = pp.tile([128, KC], I32, tag="gidx")
            nc.sync.dma_start(out=gidx_sb[:], in_=gidx_d[:])

            kT = pp.tile([128, FC, N], F32R, tag="kT")
            qn = pp.tile([128, FC, N], F32R, tag="qn")
            v_aug = pp.tile([128, NT, HPC, DH + 1], BF16, tag="vaug")
            nc.vector.tensor_copy(
                v_aug[:, :, :, DH : DH + 1],
                ones_col32[:].to_broadcast([128, NT, HPC, 1]),
            )
            o_sb = pp.tile([128, FC, N], BF16, tag="osb")
            rmsk_col = pp.tile([128, NT], F32, tag="rmskcol")

            # ================= PHASE A: projections + rope =================
            with (
                tc.tile_pool(name="pa", bufs=1) as pa,
                tc.tile_pool(name="xtp", bufs=9) as pxt,
                tc.tile_pool(name="pwa", bufs=3) as pwa,
                tc.tile_pool(name="psA", bufs=5, space="PSUM") as psA,
                tc.tile_pool(name="psV", bufs=2, space="PSUM") as psV,
                tc.tile_pool(name="psS", bufs=1, space="PSUM") as psS,
            ):
                # pair-swap permutation matrix: psw[p, 2f+e] = 1 iff p == 2f+1-e
                psw32 = pa.tile([128, 128], F32, tag="psw32")
                nc.gpsimd.memset(psw32[:], 0.0)
                nc.gpsimd.affine_select(
                    out=psw32[:].rearrange("p (f e) -> p f e", e=2),
                    in_=psw32[:].rearrange("p (f e) -> p f e", e=2),
                    compare_op=mybir.AluOpType.not_equal,
                    fill=1.0,
                    base=-1,
                    pattern=[[-2, 64], [1, 2]],
                    channel_multiplier=1,
                )
                psw = pa.tile([128, 128], F32R, tag="psw")
                nc.vector.tensor_copy(psw[:], psw32[:])

                wq_sb = pa.tile([128, KC, FPC], F32R, tag="wq")
                wk_sb = pa.tile([128, KC, FPC], F32R, tag="wk")
                wv_sb = pa.tile([128, KC, FPC], F32R, tag="wv")
                for w_sb, w_d in ((wq_sb, wq_d), (wk_sb, wk_d), (wv_sb, wv_d)):
                    nc.gpsimd.dma_start(
                        out=w_sb[:], in_=w_d[:].rearrange("(kc p) f -> p kc f", p=128)
                    )
                cos_sb = pa.tile([128, N], F32, tag="cos")
                sin_sb = pa.tile([128, N], F32, tag="sin")
                nc.sync.dma_start(out=cos_sb[:], in_=cos_d[:])
                nc.sync.dma_start(out=sin_sb[:], in_=sin_d[:])
                qpre = pa.tile([128, FC, N], F32, tag="qpre")
                # row-vector stripes; engine ops need base partition 0/32/64,
                # so q lives at partition 0 and k at partition 32 of each.
                ssqp = pa.tile([64, N], F32, tag="ssqp")    # ssq partials
                sq2 = pa.tile([64, N], F32, tag="sq2")      # allreduced ssq
                sqr = pa.tile([64, N], F32, tag="sqr")      # sqrt results
                rinv = pa.tile([64, N], F32, tag="rinv")    # 0: rq_inv, 32: rk8

                for j in range(NJ):
                    jsl = slice(j * 512, (j + 1) * 512)
                    xt = [
                        pxt.tile([128, 512], F32R, tag="xt", name=f"xt{j}_{i}")
                        for i in range(KC)
                    ]
                    for kc in range(KC):
                        nc.gpsimd.dma_start(
                            out=xt[kc][:], in_=xT[kc * 128 : (kc + 1) * 128, jsl]
                        )

                    for ti, (w_sb, wcol, dest) in enumerate((
                        (wq_sb, wqc_sb, qpre),
                        (wk_sb, wkc_sb, kT),
                    )):
                        ssq_ps = psS.tile([1, 512], F32, tag="ssq", name=f"ssq{j}")
                        for fc in range(FC):
                            fsl = slice(fc * 128, (fc + 1) * 128)
                            prj = psA.tile([128, 512], F32, tag="proj", name=f"prj{j}{fc}")
                            for kc in range(KC):
                                nc.tensor.matmul(
                                    prj[:],
                                    w_sb[:, kc, fsl],
                                    xt[kc][:],
                                    start=(kc == 0),
                                    stop=(kc == KC - 1),
                                )
                            # plain eviction (pre-norm-weight values)
                            qw = pwa.tile([128, 512], F32, tag="qw", name=f"qw{j}{fc}")
                            nc.scalar.activation(qw[:], prj[:], AF.Copy)
                            # sum-of-squares partial (DVE: only one PSUM input)
                            q2 = pwa.tile([128, 512], F32R, tag="q2", name=f"q2_{j}{fc}")
                            nc.vector.tensor_mul(q2[:], prj[:], qw[:])
                            nc.tensor.matmul(
                                ssq_ps[:],
                                ones_col_r[:],
                                q2[:],
                                start=(fc == 0),
                                stop=(fc == FC - 1),
                            )
                            # rope with norm weight folded in:
                            #   tcos = (qw * w) * cos, tsin = (qw * w) * sin'
                            mul = mybir.AluOpType.mult
                            tcos = pwa.tile([128, 512], F32, tag="tcos", name=f"tc{j}{fc}")
                            nc.vector.scalar_tensor_tensor(
                                tcos[:], qw[:], wcol[:, fc : fc + 1], cos_sb[:, jsl],
                                op0=mul, op1=mul,
                            )
                            tsin = pwa.tile([128, 512], F32R, tag="tsin", name=f"ts{j}{fc}")
                            nc.vector.scalar_tensor_tensor(
                                tsin[:], qw[:], wcol[:, fc : fc + 1], sin_sb[:, jsl],
                                op0=mul, op1=mul,
                            )
                            swp = psA.tile([128, 512], F32, tag="proj", name=f"sw{j}{fc}")
                            nc.tensor.matmul(
                                swp[:], psw[:], tsin[:], start=True, stop=True
                            )
                            nc.vector.tensor_add(dest[:, fc, jsl], tcos[:], swp[:])
                        nc.scalar.activation(
                            ssqp[32 * ti : 32 * ti + 1, jsl], ssq_ps[:], AF.Copy
                        )

                    # v projection: [n, f] orientation, into augmented layout
                    for ntl in range(4):
                        nt = j * 4 + ntl
                        nsl = slice(ntl * 128, (ntl + 1) * 128)
                        vps = psV.tile([128, FPC], F32, tag="v", name=f"v{nt}")
                        for kc in range(KC):
                            nc.tensor.matmul(
                                vps[:],
                                xt[kc][:, nsl],
                                wv_sb[:, kc, :],
                                start=(kc == 0),
                                stop=(kc == KC - 1),
                            )
                        nc.scalar.activation(
                            v_aug[:, nt, :, 0:DH],
                            vps[:].rearrange("p (h f) -> p h f", f=DH),
                            AF.Copy,
                        )

                # ---- RMSNorm allreduce + scalars --------------------------
                nc.sync.dma_start(out=ssq_in[0:1, :], in_=ssqp[0:1, :])
                nc.sync.dma_start(out=ssq_in[1:2, :], in_=ssqp[32:33, :])
                nc.gpsimd.collective_compute(
                    "AllReduce",
                    mybir.AluOpType.add,
                    replica_groups=GROUPS4,
                    ins=[ssq_in[:]],
                    outs=[ssq_out[:]],
                )
                nc.sync.dma_start(out=sq2[0:1, :], in_=ssq_out[0:1, :])
                nc.sync.dma_start(out=sq2[32:33, :], in_=ssq_out[1:2, :])

                # q: rms_q_inv = 1/sqrt(ssq/D + eps)
                nc.scalar.activation(
                    sqr[0:1, :], sq2[0:1, :], AF.Sqrt, scale=1.0 / D,
                    bias=eps_both[0:1, :],
                )
                rq_inv = rinv[0:1, :]
                nc.vector.reciprocal(rq_inv, sqr[0:1, :])
                # k: rk8 = 1/(8*sqrt(ssq/D + eps)) (score scale folded in)
                nc.scalar.activation(
                    sqr[32:33, :], sq2[32:33, :], AF.Sqrt, scale=64.0 / D,
                    bias=eps_both[32:33, :],
                )
                rk8_row = rinv[32:33, :]
                nc.vector.reciprocal(rk8_row, sqr[32:33, :])
                # k-scales into column form [128, 16] via 16 rank-1 matmuls
                # (fp32 matmuls, N=1 each -> negligible)
                rk_ps = psV.tile([128, NT], F32, tag="v")
                for t in range(NT):
                    nc.tensor.matmul(
                        rk_ps[:, t : t + 1],
                        rinv[32:33, t * 128 : (t + 1) * 128],
                        one1_f32[32:33, :],
                        start=True,
                        stop=True,
                    )
                nc.scalar.activation(rmsk_col[:], rk_ps[:], AF.Copy)
                # qn = qpre * broadcast(rq_inv)
                bq_bcast = pa.tile([128, N], F32, tag="bqb")
                nc.gpsimd.partition_broadcast(bq_bcast[:], rq_inv)
                for fc in range(FC):
                    nc.vector.tensor_mul(qn[:, fc, :], qpre[:, fc, :], bq_bcast[:])

            # ============== PHASE B: attention, per head ===================
            with (
                tc.tile_pool(name="pb", bufs=1) as pb,
                tc.tile_pool(name="pwb", bufs=3) as pwb,
            ):
                # out-projection weights: loaded here so the DMA + cast overlap
                # with attention compute
                wo_sb = pb.tile([128, KC, D], BF16, tag="wo")
                nc.gpsimd.dma_start(
                    out=wo_sb[:], in_=wo_d[:].rearrange("(kc p) f -> p kc f", p=128)
                )
                bo_sb = pb.tile([1, D], BF16, tag="bo")
                nc.gpsimd.dma_start(out=bo_sb[:], in_=bo_d[:])

                with (
                    tc.tile_pool(name="ppb", bufs=4) as ppb,
                    tc.tile_pool(name="psB", bufs=2, space="PSUM") as psB,
                    tc.tile_pool(name="psOV", bufs=4, space="PSUM") as psOV,
                ):

                    for h in range(HPC):
                        ch = h // 2
                        po = 64 * (h % 2)
                        ov = [
                            psOV.tile([DH + 1, 512], F32, tag="ov", name=f"ov{h}_{i}")
                            for i in range(NJ)
                        ]
                        for mt in range(NT):
                            msl = slice(mt * 128, (mt + 1) * 128)
                            # S split into two double-buffered 2-bank halves so
                            # next-half scores overlap current-half exp
                            for hf in range(2):
                                s_ps = psB.tile(
                                    [128, N // 2], F32, tag="S",
                                    name=f"S{h}_{mt}_{hf}",
                                )
                                for j2 in range(2):
                                    j = hf * 2 + j2
                                    jsl = slice(j * 512, (j + 1) * 512)
                                    nc.tensor.matmul(
                                        s_ps[:, j2 * 512 : (j2 + 1) * 512],
                                        kT[po : po + 64, ch, msl],
                                        qn[po : po + 64, ch, jsl],
                                        start=True,
                                        stop=True,
                                    )
                                p_t = ppb.tile(
                                    [128, N // 2], BF16, tag="P",
                                    name=f"P{h}_{mt}_{hf}",
                                )
                                nc.scalar.activation(
                                    p_t[:], s_ps[:], AF.Exp,
                                    scale=rmsk_col[:, mt : mt + 1],
                                )
                                for j2 in range(2):
                                    j = hf * 2 + j2
                                    nc.tensor.matmul(
                                        ov[j][:],
                                        v_aug[:, mt, h, :],
                                        p_t[:, j2 * 512 : (j2 + 1) * 512],
                                        start=(mt == 0),
                                        stop=(mt == NT - 1),
                                    )
                        for j in range(NJ):
                            jsl = slice(j * 512, (j + 1) * 512)
                            rec = pwb.tile(
                                [DH + 1, 512], F32, tag="rec", name=f"rec{h}{j}"
                            )
                            nc.vector.reciprocal(
                                rec[DH : DH + 1, :], ov[j][DH : DH + 1, :]
                            )
                            rec0 = pwb.tile([1, 512], F32, tag="rec0", name=f"r0{h}{j}")
                            nc.sync.dma_start(
                                out=rec0[:], in_=rec[DH : DH + 1, :]
                            )
                            bcast = pwb.tile([DH, 512], F32, tag="bcast", name=f"bc{h}{j}")
                            nc.gpsimd.partition_broadcast(bcast[:], rec0[:])
                            nc.vector.tensor_mul(
                                o_sb[po : po + 64, ch, jsl], ov[j][0:DH, :], bcast[:]
                            )
                        if h % 2 == 1:
                            nc.sync.dma_start(out=ag_in[ch][:], in_=o_sb[:, ch, :])
                            nc.gpsimd.collective_compute(
                                "AllGather",
                                mybir.AluOpType.bypass,
                                replica_groups=GROUP8,
                                ins=[ag_in[ch][:]],
                                outs=[ag_out[ch][:]],
                            )

                # ========= PHASE C: slice-gather + output projection =======
                og = pb.tile([128, KC, 512], BF16, tag="og")
                for kc in range(KC):
                    nc.gpsimd.indirect_dma_start(
                        out=og[:, kc, :],
                        out_offset=None,
                        in_=ag_out[kc % FC][:].rearrange("r (j n) -> (r j) n", n=512),
                        in_offset=bass.IndirectOffsetOnAxis(
                            ap=gidx_sb[:, kc : kc + 1], axis=0
                        ),
                    )
                if dbg:
                    nc.gpsimd.dma_start(out=dbg_qn[:], in_=qn[:])
                    nc.gpsimd.dma_start(out=dbg_kt[:], in_=kT[:])
                    nc.sync.dma_start(out=dbg_va[:], in_=v_aug[:])
                    nc.sync.dma_start(out=dbg_osb[:], in_=o_sb[:])
                    nc.sync.dma_start(out=dbg_og[:], in_=og[:])
                with tc.tile_pool(name="psC", bufs=4, space="PSUM") as psC:
                    for ntl in range(4):
                        for dc in range(2):
                            dsl = slice(dc * 512, (dc + 1) * 512)
                            yps = psC.tile(
                                [128, 512], F32, tag="y", name=f"y{ntl}{dc}"
                            )
                            for kc in range(KC):
                                nc.tensor.matmul(
                                    yps[:],
                                    og[:, kc, ntl * 128 : (ntl + 1) * 128],
                                    wo_sb[:, kc, dsl],
                                    start=(kc == 0),
                                    stop=False,
                                )
                            nc.tensor.matmul(
                                yps[:],
                                ones_row_bf[:],
                                bo_sb[0:1, dsl],
                                start=False,
                                stop=True,
                            )
                            ysb = pwb.tile([128, 512], F32, tag="ysb", name=f"ysb{ntl}{dc}")
                            nc.scalar.activation(ysb[:], yps[:], AF.Copy)
                            nc.sync.dma_start(
                                out=out_d[ntl * 128 : (ntl + 1) * 128, dsl],
                                in_=ysb[:],
                            )

    nc.compile()
    return nc


def _rope_tables():
    """cos/sin tables matching the reference's f32 angle computation.

    C[d, n] = cos(n * theta[d//2]);  Ssw[2i] = +sin, Ssw[2i+1] = -sin
    (Ssw is the swapped-operand multiplier: rope = x*C + swap(x*Ssw)).
    Tiled x2 along partitions to cover a 2-head (128-row) chunk.
    """
    i2 = np.arange(0, DH, 2, dtype=np.float32)
    theta = (1.0 / (10000.0 ** (i2 / DH))).astype(np.float32)  # (32,)
    ang = np.arange(N, dtype=np.float32)[:, None] * theta[None, :]  # (N, 32) f32
    cos = np.cos(ang.astype(np.float64)).astype(np.float32).T  # (32, N)
    sin = np.sin(ang.astype(np.float64)).astype(np.float32).T
    cos_d = np.repeat(cos, 2, axis=0)  # (64, N)
    ssw = np.repeat(sin, 2, axis=0)
    ssw[1::2, :] *= -1.0
    cos_t = np.tile(cos_d, (2, 1)).astype(np.float32)  # (128, N)
    sin_t = np.tile(ssw, (2, 1)).astype(np.float32)
    return cos_t, sin_t


def kernel(x, Wq, Wkv, norm_q_w, norm_k_w, Wo, bo, _trace=False, _dbg=False):
    global _CACHED_NC
    x = np.ascontiguousarray(np.asarray(x, dtype=np.float32))
    Wq = np.asarray(Wq, dtype=np.float32)
    Wkv = np.asarray(Wkv, dtype=np.float32)
    norm_q_w = np.asarray(norm_q_w, dtype=np.float32)
    norm_k_w = np.asarray(norm_k_w, dtype=np.float32)
    Wo = np.asarray(Wo, dtype=np.float32)
    bo = np.asarray(bo, dtype=np.float32)

    cos_t, sin_t = _rope_tables()
    if _dbg:
        nc = build(dbg=True)
    else:
        if _CACHED_NC is None:
            _CACHED_NC = build()
        nc = _CACHED_NC

    in_maps = []
    for c in range(CORES):
        b, g = c // 4, c % 4
        fsl = slice(g * FPC, (g + 1) * FPC)
        gidx = np.empty((128, KC), dtype=np.int32)
        p = np.arange(128)
        for kc in range(KC):
            gidx[:, kc] = (128 * (4 * b + kc // FC) + p) * NJ + g
        in_maps.append(
            {
                "xT": np.ascontiguousarray(x[b].T),
                "wq": np.ascontiguousarray(Wq[:, fsl]),
                "wk": np.ascontiguousarray(Wkv[:, fsl]),
                "wv": np.ascontiguousarray(Wkv[:, D + g * FPC : D + (g + 1) * FPC]),
                "wo": Wo,
                "bo": bo.reshape(1, D),
                "wqc": np.ascontiguousarray(norm_q_w[fsl].reshape(FC, 128).T),
                "wkc": np.ascontiguousarray(norm_k_w[fsl].reshape(FC, 128).T),
                "cos_t": cos_t,
                "sin_t": sin_t,
                "gidx": gidx,
            }
        )

    res = run_bass_kernel_spmd(nc, in_maps, list(range(CORES)), trace=_trace)
    out = np.empty((B, N, D), dtype=np.float32)
    for c in range(CORES):
        b, g = c // 4, c % 4
        out[b, g * 512 : (g + 1) * 512, :] = res.results[c]["out"]
    if _trace or _dbg:
        return out, res
    return out



# revision 23
# speedup vs baseline: 1.2323x; 1.2323x over previous
"""Distributed Bass kernel for nn_Attention (B=2, N=2048, D=1024, H=16, DH=64) on 8 trn2 cores.

Sharding: data-parallel over batch (cores 0-3 -> b=0, 4-7 -> b=1), tensor-parallel
over heads (4 heads / 256 inner features per core).  v2 design (bf16 everywhere):
  all matmuls bf16 (weights/x pre-cast+rearranged on host, fp32 PSUM accumulate),
  q/k projections + rope first, ssq AllReduce overlapped with v projections,
  RMSNorm scales via ACT ln/exp (single activation table set for whole kernel),
  attention scores col-packed 2x via tile_position quadrants (DH=64),
  softmax denominator via ones-row in v, fast-approx reciprocal,
  8-core AllGather per 2-head chunk (bf16), out-projection ordered so the
  second AllGather overlaps the first half of the output matmuls.
Host assembles the (2, 2048, 1024) output from the 8 (512, 1024) shards.
"""
import os
import sys

for _p in ("/opt/trn_rl_repo", "/root/.axon_site/_ro/trn_rl_repo"):
    if os.path.isdir(_p) and _p not in sys.path:
        sys.path.insert(0, _p)

import numpy as np
import ml_dtypes
import concourse.bass as bass
import concourse.mybir as mybir
import concourse.tile as tile
from concourse import bacc
from concourse.bass_utils import run_bass_kernel_spmd

dt = mybir.dt
AF = mybir.ActivationFunctionType
F32, BF16, I32 = dt.float32, dt.bfloat16, dt.int32
BF = ml_dtypes.bfloat16

B, N, D = 2, 2048, 1024
H, DH = 16, 64
HPC = 4            # heads per core
FPC = HPC * DH     # 256 inner features per core
KC = D // 128      # 8 contraction chunks
FC = FPC // 128    # 2 feature chunks per core
NJ = N // 512      # 4 gather chunks (AllGather slice granularity)
NT = N // 128      # 16 m-tiles
NJ2 = N // 1024    # 2 compute chunks
EPS = 1e-6
CORES = 8
GROUPS4 = [[0, 1, 2, 3], [4, 5, 6, 7]]
GROUP8 = [list(range(CORES))]

_CACHED_NC = None


def build(dbg=False):
    nc = bacc.Bacc("TRN2", target_bir_lowering=False, debug=False, num_devices=CORES)

    xT = nc.dram_tensor("xT", [128, KC, N], BF16, kind="ExternalInput")
    wq_d = nc.dram_tensor("wq", [128, KC, FPC], BF16, kind="ExternalInput")
    wk_d = nc.dram_tensor("wk", [128, KC, FPC], BF16, kind="ExternalInput")
    wv_d = nc.dram_tensor("wv", [128, KC, FPC], BF16, kind="ExternalInput")
    wo_d = nc.dram_tensor("wo", [128, KC, D], BF16, kind="ExternalInput")
    bo_d = nc.dram_tensor("bo", [1, D], BF16, kind="ExternalInput")
    wqc_d = nc.dram_tensor("wqc", [128, FC], BF16, kind="ExternalInput")
    wkc_d = nc.dram_tensor("wkc", [128, FC], BF16, kind="ExternalInput")
    cos_d = nc.dram_tensor("cos_t", [128, N], BF16, kind="ExternalInput")
    sin_d = nc.dram_tensor("sin_t", [128, N], BF16, kind="ExternalInput")
    gidx_d = nc.dram_tensor("gidx", [128, KC], I32, kind="ExternalInput")
    out_d = nc.dram_tensor("out", [512, D], F32, kind="ExternalOutput")

    if dbg:
        dbg_qn = nc.dram_tensor("dbg_qn", [128, FC, N], BF16, kind="ExternalOutput")
        dbg_kt = nc.dram_tensor("dbg_kt", [128, FC, N], BF16, kind="ExternalOutput")
        dbg_ri = nc.dram_tensor("dbg_ri", [33, N], F32, kind="ExternalOutput")
        dbg_rm = nc.dram_tensor("dbg_rm", [128, NT], F32, kind="ExternalOutput")
        dbg_va = nc.dram_tensor(
            "dbg_va", [128, NT, HPC, DH + 1], BF16, kind="ExternalOutput"
        )
        dbg_osb = nc.dram_tensor("dbg_osb", [128, FC, N], BF16, kind="ExternalOutput")
        dbg_og = nc.dram_tensor("dbg_og", [128, KC, 512], BF16, kind="ExternalOutput")
        dbg_pt = nc.dram_tensor("dbg_pt", [128, 1024], BF16, kind="ExternalOutput")
        dbg_rec = nc.dram_tensor("dbg_rec", [HPC * 2, 1024], F32, kind="ExternalOutput")

    # collective bounce buffers
    ssq_in = nc.dram_tensor("ssq_in", [2, N], F32)
    ssq_out = nc.dram_tensor("ssq_out", [2, N], F32)
    ag_in = [nc.dram_tensor(f"ag_in{c}", [128, N], BF16) for c in range(FC)]
    ag_out = [
        nc.dram_tensor(f"ag_out{c}", [CORES * 128, N], BF16, addr_space="Shared")
        for c in range(FC)
    ]

    with tile.TileContext(nc) as tc:
        with tc.tile_pool(name="persist", bufs=1) as pp:
            # ---- constants ------------------------------------------------
            ones_col32 = pp.tile([128, 1], F32, tag="onesc32")
            nc.gpsimd.memset(ones_col32[:], 1.0)
            ones_col_bf = pp.tile([128, 1], BF16, tag="onescbf")
            nc.vector.tensor_copy(ones_col_bf[:], ones_col32[:])
            ones_row32 = pp.tile([1, 128], F32, tag="onesr32")
            nc.gpsimd.memset(ones_row32[:], 1.0)
            ones_row_bf = pp.tile([1, 128], BF16, tag="onesrbf")
            nc.vector.tensor_copy(ones_row_bf[:], ones_row32[:])
            one1_f32 = pp.tile([33, 1], F32, tag="one1")
            nc.gpsimd.memset(one1_f32[:], 1.0)
            # activation bias values at consumer base partitions
            eps_t = pp.tile([33, 1], F32, tag="eps")
            nc.gpsimd.memset(eps_t[0:1, :], EPS)
            nc.gpsimd.memset(eps_t[32:33, :], EPS)
            bexp_t = pp.tile([33, 1], F32, tag="bexp")
            nc.gpsimd.memset(bexp_t[0:1, :], 0.0)
            nc.gpsimd.memset(bexp_t[32:33, :], -float(np.log(8.0)))

            wqc_sb = pp.tile([128, FC], BF16, tag="wqc")
            wkc_sb = pp.tile([128, FC], BF16, tag="wkc")
            nc.sync.dma_start(out=wqc_sb[:], in_=wqc_d[:])
            nc.sync.dma_start(out=wkc_sb[:], in_=wkc_d[:])
            gidx_sb = pp.tile([128, KC], I32, tag="gidx")
            nc.sync.dma_start(out=gidx_sb[:], in_=gidx_d[:])

            # ---- big persistent tensors ----------------------------------
            xsb = pp.tile([128, KC, N], BF16, tag="xsb")
            for j in range(NJ2):
                jsl = slice(j * 1024, (j + 1) * 1024)
                nc.sync.dma_start(out=xsb[:, :, jsl], in_=xT[:, :, jsl])
            wq_sb = pp.tile([128, KC, FPC], BF16, tag="wq")
            wk_sb = pp.tile([128, KC, FPC], BF16, tag="wk")
            wv_sb = pp.tile([128, KC, FPC], BF16, tag="wv")
            nc.sync.dma_start(out=wq_sb[:], in_=wq_d[:])
            nc.sync.dma_start(out=wk_sb[:], in_=wk_d[:])
            nc.sync.dma_start(out=wv_sb[:], in_=wv_d[:])
            cos_sb = pp.tile([128, N], BF16, tag="cos")
            sin_sb = pp.tile([128, N], BF16, tag="sin")
            nc.sync.dma_start(out=cos_sb[:], in_=cos_d[:])
            nc.sync.dma_start(out=sin_sb[:], in_=sin_d[:])
            wo_sb = pp.tile([128, KC, D], BF16, tag="wo")
            nc.gpsimd.dma_start(out=wo_sb[:], in_=wo_d[:])
            bo_sb = pp.tile([1, D], BF16, tag="bo")
            nc.gpsimd.dma_start(out=bo_sb[:], in_=bo_d[:])

            kT = pp.tile([128, FC, N], BF16, tag="kT")
            qn = pp.tile([128, FC, N], BF16, tag="qn")
            v_aug = pp.tile([128, NT, HPC, DH + 1], BF16, tag="vaug")
            nc.vector.tensor_copy(
                v_aug[:, :, :, DH : DH + 1],
                ones_col32[:].to_broadcast([128, NT, HPC, 1]),
            )
            o_sb = pp.tile([128, FC, N], BF16, tag="osb")
            rmsk_col = pp.tile([128, NT], F32, tag="rmskcol")
            og = pp.tile([128, KC, 512], BF16, tag="og")

            # ================= PHASE A: q/k projections + rope =============
            with (
                tc.tile_pool(name="pa", bufs=1) as pa,
                tc.tile_pool(name="pwa", bufs=4) as pwa,
            ):
                # pair-swap permutation matrix: psw[p, 2f+e] = 1 iff p == 2f+1-e
                psw32 = pa.tile([128, 128], F32, tag="psw32")
                nc.gpsimd.memset(psw32[:], 0.0)
                nc.gpsimd.affine_select(
                    out=psw32[:].rearrange("p (f e) -> p f e", e=2),
                    in_=psw32[:].rearrange("p (f e) -> p f e", e=2),
                    compare_op=mybir.AluOpType.not_equal,
                    fill=1.0,
                    base=-1,
                    pattern=[[-2, 64], [1, 2]],
                    channel_multiplier=1,
                )
                psw = pa.tile([128, 128], BF16, tag="psw")
                nc.vector.tensor_copy(psw[:], psw32[:])

                qpre = pa.tile([128, FC, N], BF16, tag="qpre")
                # row-vector stripes at base partitions 0 (q) and 32 (k)
                ssqp = pa.tile([33, N], F32, tag="ssqp")
                sq2 = pa.tile([33, N], F32, tag="sq2")
                lnv = pa.tile([33, N], F32, tag="lnv")
                rinv = pp.tile([33, N], F32, tag="rinv")
                bq = pa.tile([128, N], F32, tag="bq")

                mul = mybir.AluOpType.mult
                with (
                    tc.tile_pool(name="psA", bufs=3, space="PSUM") as psA,
                    tc.tile_pool(name="psSw", bufs=2, space="PSUM") as psSw,
                    tc.tile_pool(name="psS", bufs=2, space="PSUM") as psS,
                ):
                  for j in range(NJ):
                    jsl = slice(j * 512, (j + 1) * 512)
                    for ti, (w_sb, wcol, dest) in enumerate((
                        (wq_sb, wqc_sb, qpre),
                        (wk_sb, wkc_sb, kT),
                    )):
                        ssq_ps = psS.tile([1, 512], F32, tag="ssq", name=f"ssq{j}{ti}")
                        for fc in range(FC):
                            fsl = slice(fc * 128, (fc + 1) * 128)
                            prj = psA.tile(
                                [128, 512], F32, tag="proj", name=f"prj{j}{ti}{fc}"
                            )
                            for kc in range(KC):
                                nc.tensor.matmul(
                                    prj[:],
                                    w_sb[:, kc, fsl],
                                    xsb[:, kc, jsl],
                                    start=(kc == 0),
                                    stop=(kc == KC - 1),
                                )
                            qw = pwa.tile([128, 512], BF16, tag="qw", name=f"qw{j}{ti}{fc}")
                            nc.scalar.activation(qw[:], prj[:], AF.Copy)
                            # sum-of-squares partial (DVE: only one PSUM input)
                            q2 = pwa.tile([128, 512], BF16, tag="q2", name=f"q2_{j}{ti}{fc}")
                            nc.vector.tensor_mul(q2[:], prj[:], qw[:])
                            nc.tensor.matmul(
                                ssq_ps[:],
                                ones_col_bf[:],
                                q2[:],
                                start=(fc == 0),
                                stop=(fc == FC - 1),
                            )
                            # rope with norm weight folded in
                            tcos = pwa.tile([128, 512], BF16, tag="tcos", name=f"tc{j}{ti}{fc}")
                            nc.vector.scalar_tensor_tensor(
                                tcos[:], qw[:], wcol[:, fc : fc + 1], cos_sb[:, jsl],
                                op0=mul, op1=mul,
                            )
                            tsin = pwa.tile([128, 512], BF16, tag="tsin", name=f"ts{j}{ti}{fc}")
                            nc.vector.scalar_tensor_tensor(
                                tsin[:], qw[:], wcol[:, fc : fc + 1], sin_sb[:, jsl],
                                op0=mul, op1=mul,
                            )
                            swp = psSw.tile([128, 512], F32, tag="swp", name=f"sw{j}{ti}{fc}")
                            nc.tensor.matmul(swp[:], psw[:], tsin[:], start=True, stop=True)
                            nc.vector.tensor_add(dest[:, fc, jsl], tcos[:], swp[:])
                        nc.scalar.activation(
                            ssqp[32 * ti : 32 * ti + 1, jsl], ssq_ps[:], AF.Copy
                        )

                # ---- RMSNorm allreduce (overlaps v projections below) -----
                nc.sync.dma_start(out=ssq_in[0:1, :], in_=ssqp[0:1, :])
                nc.sync.dma_start(out=ssq_in[1:2, :], in_=ssqp[32:33, :])
                nc.gpsimd.collective_compute(
                    "AllReduce",
                    mybir.AluOpType.add,
                    replica_groups=GROUPS4,
                    ins=[ssq_in[:]],
                    outs=[ssq_out[:]],
                )
                nc.sync.dma_start(out=sq2[0:1, :], in_=ssq_out[0:1, :])
                nc.sync.dma_start(out=sq2[32:33, :], in_=ssq_out[1:2, :])

                # rq = rsqrt(ssq/D + eps) = exp(-0.5*ln(ssq/D + eps))
                # rk8 = rq_k / 8      (score scale folded in, bias = -ln 8)
                for ti in range(2):
                    r = slice(32 * ti, 32 * ti + 1)
                    nc.scalar.activation(
                        lnv[r, :], sq2[r, :], AF.Ln, scale=1.0 / D, bias=eps_t[r, :]
                    )
                    nc.scalar.activation(
                        rinv[r, :], lnv[r, :], AF.Exp, scale=-0.5, bias=bexp_t[r, :]
                    )

                # ---- v projections (overlap the AllReduce) ----------------
                with tc.tile_pool(name="psV", bufs=4, space="PSUM") as psV:
                    for nt in range(NT):
                        nsl = slice((nt % 8) * 128, (nt % 8 + 1) * 128)
                        jsl = slice((nt // 8) * 1024, (nt // 8) * 1024 + 1024)
                        vps = psV.tile([128, FPC], F32, tag="v", name=f"v{nt}")
                        for kc in range(KC):
                            nc.tensor.matmul(
                                vps[:],
                                xsb[:, kc, jsl][:, nsl],
                                wv_sb[:, kc, :],
                                start=(kc == 0),
                                stop=(kc == KC - 1),
                            )
                        nc.scalar.activation(
                            v_aug[:, nt, :, 0:DH],
                            vps[:].rearrange("p (h f) -> p h f", f=DH),
                            AF.Copy,
                        )

                    # k-scales into column form [128, 16] via 16 rank-1 matmuls
                    rk_ps = psV.tile([128, NT], F32, tag="v", name="rkps")
                    for t in range(NT):
                        nc.tensor.matmul(
                            rk_ps[:, t : t + 1],
                            rinv[32:33, t * 128 : (t + 1) * 128],
                            one1_f32[32:33, :],
                            start=True,
                            stop=True,
                        )
                    nc.vector.tensor_copy(rmsk_col[:], rk_ps[:])

                # qn = qpre * broadcast(rq)
                nc.gpsimd.partition_broadcast(bq[:], rinv[0:1, :])
                for fc in range(FC):
                    nc.vector.tensor_mul(qn[:, fc, :], qpre[:, fc, :], bq[:])

            # ============== PHASE B: attention, per head ===================
            with (
                tc.tile_pool(name="pwb", bufs=3) as pwb,
                tc.tile_pool(name="ppb", bufs=3) as ppb,
                tc.tile_pool(name="psB", bufs=2, space="PSUM") as psB,
                tc.tile_pool(name="psOV", bufs=2, space="PSUM") as psOV,
            ):
                for h in range(HPC):
                    ch = h // 2
                    po = 64 * (h % 2)
                    ov = [
                        psOV.tile([DH + 1, 1024], F32, tag="ov", name=f"ov{h}_{i}")
                        for i in range(NJ2)
                    ]
                    units = [(mt, hf) for mt in range(NT) for hf in range(NJ2)]

                    def emit_scores(k):
                        mt, hf = units[k]
                        s_ps = psB.tile(
                            [128, 1024], F32, tag="S", name=f"S{h}_{mt}_{hf}"
                        )
                        # col-packed pairs: m-halves go to array col groups
                        # 0/64 concurrently (tile_position auto-derived)
                        for j2 in range(2):
                            jsl = slice(
                                hf * 1024 + j2 * 512, hf * 1024 + j2 * 512 + 512
                            )
                            for mh in range(2):
                                msl = slice(
                                    mt * 128 + mh * 64, mt * 128 + mh * 64 + 64
                                )
                                nc.tensor.matmul(
                                    s_ps[mh * 64 : (mh + 1) * 64,
                                         j2 * 512 : (j2 + 1) * 512],
                                    kT[po : po + 64, ch, msl],
                                    qn[po : po + 64, ch, jsl],
                                    start=True,
                                    stop=True,
                                )
                        return s_ps

                    def emit_expov(k, s_ps):
                        mt, hf = units[k]
                        p_t = ppb.tile(
                            [128, 1024], BF16, tag="P", name=f"P{h}_{mt}_{hf}"
                        )
                        nc.scalar.activation(
                            p_t[:], s_ps[:], AF.Exp,
                            scale=rmsk_col[:, mt : mt + 1],
                        )
                        if dbg and h == 0 and mt == 0 and hf == 0:
                            nc.sync.dma_start(out=dbg_pt[:], in_=p_t[:])
                        for j2 in range(2):
                            nc.tensor.matmul(
                                ov[hf][:, j2 * 512 : (j2 + 1) * 512],
                                v_aug[:, mt, h, :],
                                p_t[:, j2 * 512 : (j2 + 1) * 512],
                                start=(mt == 0),
                                stop=(mt == NT - 1),
                            )

                    # software pipeline: scores(k+1) is emitted before ov(k)
                    # so the in-order PE queue never stalls on the exp
                    prev = None
                    for k in range(len(units)):
                        cur = emit_scores(k)
                        if prev is not None:
                            emit_expov(k - 1, prev)
                        prev = cur
                    emit_expov(len(units) - 1, prev)

                    for hf in range(NJ2):
                        jsl = slice(hf * 1024, (hf + 1) * 1024)
                        # rec = 1/den via ACT ln->exp (same table set as the
                        # softmax exp, reads the PSUM denominator row directly)
                        dln = pwb.tile([65, 1024], F32, tag="dln", name=f"dl{h}{hf}")
                        nc.scalar.activation(
                            dln[DH : DH + 1, :], ov[hf][DH : DH + 1, :], AF.Ln
                        )
                        rec = pwb.tile([65, 1024], F32, tag="rec", name=f"rec{h}{hf}")
                        nc.scalar.activation(
                            rec[DH : DH + 1, :], dln[DH : DH + 1, :], AF.Exp,
                            scale=-1.0,
                        )
                        rec0 = pwb.tile([1, 1024], F32, tag="rec0", name=f"r0{h}{hf}")
                        nc.sync.dma_start(out=rec0[:], in_=rec[DH : DH + 1, :])
                        if dbg:
                            nc.sync.dma_start(
                                out=dbg_rec[2 * h + hf : 2 * h + hf + 1, :], in_=rec0[:]
                            )
                        bcast = pwb.tile([DH, 1024], F32, tag="bcast", name=f"bc{h}{hf}")
                        nc.gpsimd.partition_broadcast(bcast[:], rec0[:])
                        nc.vector.tensor_mul(
                            o_sb[po : po + 64, ch, jsl], ov[hf][0:DH, :], bcast[:]
                        )
                    if h % 2 == 1:
                        nc.sync.dma_start(out=ag_in[ch][:], in_=o_sb[:, ch, :])
                        nc.gpsimd.collective_compute(
                            "AllGather",
                            mybir.AluOpType.bypass,
                            replica_groups=GROUP8,
                            ins=[ag_in[ch][:]],
                            outs=[ag_out[ch][:]],
                        )
                        # gather this chunk's slices as soon as the AG lands
                        # (ch 0 overlaps heads 2-3; ch 1 overlaps nothing)
                        for kc in range(ch, KC, FC):
                            nc.gpsimd.indirect_dma_start(
                                out=og[:, kc, :],
                                out_offset=None,
                                in_=ag_out[ch][:].rearrange(
                                    "r (j n) -> (r j) n", n=512
                                ),
                                in_offset=bass.IndirectOffsetOnAxis(
                                    ap=gidx_sb[:, kc : kc + 1], axis=0
                                ),
                            )

            if dbg:
                nc.gpsimd.dma_start(out=dbg_qn[:], in_=qn[:])
                nc.gpsimd.dma_start(out=dbg_kt[:], in_=kT[:])
                nc.sync.dma_start(out=dbg_ri[:], in_=rinv[:])
                nc.sync.dma_start(out=dbg_rm[:], in_=rmsk_col[:])
                nc.sync.dma_start(out=dbg_va[:], in_=v_aug[:])
                nc.sync.dma_start(out=dbg_osb[:], in_=o_sb[:])
                nc.sync.dma_start(out=dbg_og[:], in_=og[:])

            # ========= PHASE C: output projection ==========================
            # even (ch 0) contractions first: they are ready while the ch 1
            # AllGather is still in flight
            kc_order = list(range(0, KC, 2)) + list(range(1, KC, 2))
            with (
                tc.tile_pool(name="pc", bufs=2) as pc,
                tc.tile_pool(name="psC", bufs=2, space="PSUM") as psC,
            ):
                for ntl in range(4):
                    yps = psC.tile([128, D], F32, tag="y", name=f"y{ntl}")
                    for dc in range(2):
                        dsl = slice(dc * 512, (dc + 1) * 512)
                        for i, kc in enumerate(kc_order):
                            nc.tensor.matmul(
                                yps[:, dsl],
                                og[:, kc, ntl * 128 : (ntl + 1) * 128],
                                wo_sb[:, kc, dsl],
                                start=(i == 0),
                                stop=False,
                            )
                        nc.tensor.matmul(
                            yps[:, dsl],
                            ones_row_bf[:],
                            bo_sb[:, dsl],
                            start=False,
                            stop=True,
                        )
                    ysb = pc.tile([128, D], F32, tag="ysb", name=f"ysb{ntl}")
                    nc.vector.tensor_copy(ysb[:], yps[:])
                    nc.sync.dma_start(
                        out=out_d[ntl * 128 : (ntl + 1) * 128, :], in_=ysb[:]
                    )

    nc.compile()
    return nc


def _rope_tables():
    """cos/sin tables matching the reference's f32 angle computation.

    C[d, n] = cos(n * theta[d//2]);  Ssw[2i] = +sin, Ssw[2i+1] = -sin
    (Ssw is the swapped-operand multiplier: rope = x*C + swap(x*Ssw)).
    Tiled x2 along partitions to cover a 2-head (128-row) chunk.
    """
    i2 = np.arange(0, DH, 2, dtype=np.float32)
    theta = (1.0 / (10000.0 ** (i2 / DH))).astype(np.float32)  # (32,)
    ang = np.arange(N, dtype=np.float32)[:, None] * theta[None, :]  # (N, 32) f32
    cos = np.cos(ang.astype(np.float64)).astype(np.float32).T  # (32, N)
    sin = np.sin(ang.astype(np.float64)).astype(np.float32).T
    cos_d = np.repeat(cos, 2, axis=0)  # (64, N)
    ssw = np.repeat(sin, 2, axis=0)
    ssw[1::2, :] *= -1.0
    cos_t = np.tile(cos_d, (2, 1)).astype(np.float32)  # (128, N)
    sin_t = np.tile(ssw, (2, 1)).astype(np.float32)
    return cos_t, sin_t


def _rearr(w):
    # [D, F] -> [128, KC, F] grouping the contraction dim into 128-row chunks
    d, f = w.shape
    return np.ascontiguousarray(
        w.reshape(KC, 128, f).transpose(1, 0, 2).astype(BF)
    )


def kernel(x, Wq, Wkv, norm_q_w, norm_k_w, Wo, bo, _trace=False, _dbg=False):
    global _CACHED_NC
    x = np.asarray(x, dtype=np.float32)
    Wq = np.asarray(Wq, dtype=np.float32)
    Wkv = np.asarray(Wkv, dtype=np.float32)
    norm_q_w = np.asarray(norm_q_w, dtype=np.float32)
    norm_k_w = np.asarray(norm_k_w, dtype=np.float32)
    Wo = np.asarray(Wo, dtype=np.float32)
    bo = np.asarray(bo, dtype=np.float32)

    cos_t, sin_t = _rope_tables()
    if _dbg:
        nc = build(dbg=True)
    else:
        if _CACHED_NC is None:
            _CACHED_NC = build()
        nc = _CACHED_NC

    in_maps = []
    for c in range(CORES):
        b, g = c // 4, c % 4
        fsl = slice(g * FPC, (g + 1) * FPC)
        gidx = np.empty((128, KC), dtype=np.int32)
        p = np.arange(128)
        for kc in range(KC):
            gidx[:, kc] = (128 * (4 * b + kc // FC) + p) * NJ + g
        in_maps.append(
            {
                "xT": _rearr(np.ascontiguousarray(x[b].T)),
                "wq": _rearr(Wq[:, fsl]),
                "wk": _rearr(Wkv[:, fsl]),
                "wv": _rearr(Wkv[:, D + g * FPC : D + (g + 1) * FPC]),
                "wo": _rearr(Wo),
                "bo": bo.reshape(1, D).astype(BF),
                "wqc": np.ascontiguousarray(
                    norm_q_w[fsl].reshape(FC, 128).T
                ).astype(BF),
                "wkc": np.ascontiguousarray(
                    norm_k_w[fsl].reshape(FC, 128).T
                ).astype(BF),
                "cos_t": cos_t.astype(BF),
                "sin_t": sin_t.astype(BF),
                "gidx": gidx,
            }
        )

    res = run_bass_kernel_spmd(nc, in_maps, list(range(CORES)), trace=_trace)
    out = np.empty((B, N, D), dtype=np.float32)
    for c in range(CORES):
        b, g = c // 4, c % 4
        out[b, g * 512 : (g + 1) * 512, :] = res.results[c]["out"]
    if _trace or _dbg:
        return out, res
    return out


# revision 27
# speedup vs baseline: 1.3898x; 1.1278x over previous
"""Distributed Bass kernel for nn_Attention (B=2, N=2048, D=1024, H=16, DH=64) on 8 trn2 cores.

Sharding: data-parallel over batch (cores 0-3 -> b=0, 4-7 -> b=1), tensor-parallel
over heads (4 heads / 256 inner features per core).  v2 design (bf16 everywhere):
  all matmuls bf16 (weights/x pre-cast+rearranged on host, fp32 PSUM accumulate),
  q/k projections + rope first, ssq AllReduce overlapped with v projections,
  RMSNorm scales via ACT ln/exp (single activation table set for whole kernel),
  attention scores col-packed 2x via tile_position quadrants (DH=64),
  softmax denominator via ones-row in v, fast-approx reciprocal,
  8-core AllGather per 2-head chunk (bf16), out-projection ordered so the
  second AllGather overlaps the first half of the output matmuls.
Host assembles the (2, 2048, 1024) output from the 8 (512, 1024) shards.
"""
import os
import sys

for _p in ("/opt/trn_rl_repo", "/root/.axon_site/_ro/trn_rl_repo"):
    if os.path.isdir(_p) and _p not in sys.path:
        sys.path.insert(0, _p)

import numpy as np
import ml_dtypes
import concourse.bass as bass
import concourse.mybir as mybir
import concourse.tile as tile
from concourse import bacc
from concourse.bass_utils import run_bass_kernel_spmd

dt = mybir.dt
AF = mybir.ActivationFunctionType
F32, BF16, I32 = dt.float32, dt.bfloat16, dt.int32
BF = ml_dtypes.bfloat16

B, N, D = 2, 2048, 1024
H, DH = 16, 64
HPC = 4            # heads per core
FPC = HPC * DH     # 256 inner features per core
KC = D // 128      # 8 contraction chunks
FC = FPC // 128    # 2 feature chunks per core
NJ = N // 512      # 4 gather chunks (AllGather slice granularity)
NT = N // 128      # 16 m-tiles
NJ2 = N // 1024    # 2 compute chunks
EPS = 1e-6
CORES = 8
GROUPS4 = [[0, 1, 2, 3], [4, 5, 6, 7]]
GROUP8 = [list(range(CORES))]

_CACHED_NC = None


def build(dbg=False):
    nc = bacc.Bacc("TRN2", target_bir_lowering=False, debug=False, num_devices=CORES)

    xT = nc.dram_tensor("xT", [128, KC, N], BF16, kind="ExternalInput")
    wq_d = nc.dram_tensor("wq", [128, KC, FPC], BF16, kind="ExternalInput")
    wk_d = nc.dram_tensor("wk", [128, KC, FPC], BF16, kind="ExternalInput")
    wv_d = nc.dram_tensor("wv", [128, KC, FPC], BF16, kind="ExternalInput")
    wo_d = nc.dram_tensor("wo", [128, KC, D], BF16, kind="ExternalInput")
    bo_d = nc.dram_tensor("bo", [1, D], BF16, kind="ExternalInput")
    wqc_d = nc.dram_tensor("wqc", [128, FC], BF16, kind="ExternalInput")
    wkc_d = nc.dram_tensor("wkc", [128, FC], BF16, kind="ExternalInput")
    cos_d = nc.dram_tensor("cos_t", [128, N], BF16, kind="ExternalInput")
    sin_d = nc.dram_tensor("sin_t", [128, N], BF16, kind="ExternalInput")
    gidx_d = nc.dram_tensor("gidx", [128, KC], I32, kind="ExternalInput")
    out_d = nc.dram_tensor("out", [512, D], F32, kind="ExternalOutput")

    if dbg:
        dbg_qn = nc.dram_tensor("dbg_qn", [128, FC, N], BF16, kind="ExternalOutput")
        dbg_kt = nc.dram_tensor("dbg_kt", [128, FC, N], BF16, kind="ExternalOutput")
        dbg_ri = nc.dram_tensor("dbg_ri", [33, N], F32, kind="ExternalOutput")
        dbg_rm = nc.dram_tensor("dbg_rm", [128, NT], F32, kind="ExternalOutput")
        dbg_va = nc.dram_tensor(
            "dbg_va", [128, NT, HPC, DH + 1], BF16, kind="ExternalOutput"
        )
        dbg_osb = nc.dram_tensor("dbg_osb", [128, FC, N], BF16, kind="ExternalOutput")
        dbg_og = nc.dram_tensor("dbg_og", [128, KC, 512], BF16, kind="ExternalOutput")
        dbg_pt = nc.dram_tensor("dbg_pt", [128, 1024], BF16, kind="ExternalOutput")
        dbg_rec = nc.dram_tensor("dbg_rec", [HPC * 2, 1024], F32, kind="ExternalOutput")

    # collective bounce buffers
    ssq_in = nc.dram_tensor("ssq_in", [2, N], F32)
    ssq_out = nc.dram_tensor("ssq_out", [2, N], F32)
    ag_in = [nc.dram_tensor(f"ag_in{c}", [128, N], BF16) for c in range(FC)]
    ag_out = [
        nc.dram_tensor(f"ag_out{c}", [CORES * 128, N], BF16, addr_space="Shared")
        for c in range(FC)
    ]

    with tile.TileContext(nc) as tc:
        with tc.tile_pool(name="persist", bufs=1) as pp:
            # ---- constants ------------------------------------------------
            ones_col32 = pp.tile([128, 1], F32, tag="onesc32")
            nc.gpsimd.memset(ones_col32[:], 1.0)
            ones_col_bf = pp.tile([128, 1], BF16, tag="onescbf")
            nc.vector.tensor_copy(ones_col_bf[:], ones_col32[:])
            ones_row32 = pp.tile([1, 128], F32, tag="onesr32")
            nc.gpsimd.memset(ones_row32[:], 1.0)
            ones_row_bf = pp.tile([1, 128], BF16, tag="onesrbf")
            nc.vector.tensor_copy(ones_row_bf[:], ones_row32[:])
            one1_f32 = pp.tile([33, 1], F32, tag="one1")
            nc.gpsimd.memset(one1_f32[:], 1.0)
            # activation bias values at consumer base partitions
            eps_t = pp.tile([33, 1], F32, tag="eps")
            nc.gpsimd.memset(eps_t[:], EPS)
            bexp_t = pp.tile([33, 1], F32, tag="bexp")
            nc.gpsimd.memset(bexp_t[:], 0.0)
            nc.gpsimd.memset(bexp_t[32:33, :], -float(np.log(8.0)))

            wqc_sb = pp.tile([128, FC], BF16, tag="wqc")
            wkc_sb = pp.tile([128, FC], BF16, tag="wkc")
            nc.sync.dma_start(out=wqc_sb[:], in_=wqc_d[:])
            nc.sync.dma_start(out=wkc_sb[:], in_=wkc_d[:])
            gidx_sb = pp.tile([128, KC], I32, tag="gidx")
            nc.sync.dma_start(out=gidx_sb[:], in_=gidx_d[:])

            # ---- big persistent tensors ----------------------------------
            xsb = pp.tile([128, KC, N], BF16, tag="xsb")
            for j in range(NJ2):
                jsl = slice(j * 1024, (j + 1) * 1024)
                nc.sync.dma_start(out=xsb[:, :, jsl], in_=xT[:, :, jsl])
            wq_sb = pp.tile([128, KC, FPC], BF16, tag="wq")
            wk_sb = pp.tile([128, KC, FPC], BF16, tag="wk")
            wv_sb = pp.tile([128, KC, FPC], BF16, tag="wv")
            nc.sync.dma_start(out=wq_sb[:], in_=wq_d[:])
            nc.sync.dma_start(out=wk_sb[:], in_=wk_d[:])
            nc.sync.dma_start(out=wv_sb[:], in_=wv_d[:])
            cos_sb = pp.tile([128, N], BF16, tag="cos")
            sin_sb = pp.tile([128, N], BF16, tag="sin")
            nc.sync.dma_start(out=cos_sb[:], in_=cos_d[:])
            nc.sync.dma_start(out=sin_sb[:], in_=sin_d[:])
            wo_sb = pp.tile([128, KC, D], BF16, tag="wo")
            nc.gpsimd.dma_start(out=wo_sb[:], in_=wo_d[:])
            bo_sb = pp.tile([1, D], BF16, tag="bo")
            nc.gpsimd.dma_start(out=bo_sb[:], in_=bo_d[:])

            kT = pp.tile([128, FC, N], BF16, tag="kT")
            qn = pp.tile([128, FC, N], BF16, tag="qn")
            v_aug = pp.tile([128, NT, HPC, DH + 1], BF16, tag="vaug")
            nc.vector.tensor_copy(
                v_aug[:, :, :, DH : DH + 1],
                ones_col32[:].to_broadcast([128, NT, HPC, 1]),
            )
            o_sb = pp.tile([128, FC, N], BF16, tag="osb")
            rmsk_col = pp.tile([128, NT], F32, tag="rmskcol")
            og = pp.tile([128, KC, 512], BF16, tag="og")

            # ================= PHASE A: q/k projections + rope =============
            with (
                tc.tile_pool(name="pa", bufs=1) as pa,
                tc.tile_pool(name="pwa", bufs=4) as pwa,
            ):
                # pair-swap permutation matrix: psw[p, 2f+e] = 1 iff p == 2f+1-e
                psw32 = pa.tile([128, 128], F32, tag="psw32")
                nc.gpsimd.memset(psw32[:], 0.0)
                nc.gpsimd.affine_select(
                    out=psw32[:].rearrange("p (f e) -> p f e", e=2),
                    in_=psw32[:].rearrange("p (f e) -> p f e", e=2),
                    compare_op=mybir.AluOpType.not_equal,
                    fill=1.0,
                    base=-1,
                    pattern=[[-2, 64], [1, 2]],
                    channel_multiplier=1,
                )
                psw = pa.tile([128, 128], BF16, tag="psw")
                nc.vector.tensor_copy(psw[:], psw32[:])

                qpre = pa.tile([128, FC, N], BF16, tag="qpre")
                # row-vector stripes at base partitions 0 (q) and 32 (k)
                ssqp = pa.tile([33, N], F32, tag="ssqp")
                sq2 = pa.tile([33, N], F32, tag="sq2")
                lnv = pa.tile([33, N], F32, tag="lnv")
                rinv = pp.tile([33, N], F32, tag="rinv")
                bq = pa.tile([128, N], F32, tag="bq")
                # rows 1-31 are never written by the ssq path but are read by
                # the combined [33, N] ln/exp below; keep them finite
                nc.gpsimd.memset(sq2[:], 1.0)

                mul = mybir.AluOpType.mult
                with (
                    tc.tile_pool(name="psA", bufs=3, space="PSUM") as psA,
                    tc.tile_pool(name="psSw", bufs=2, space="PSUM") as psSw,
                    tc.tile_pool(name="psS", bufs=2, space="PSUM") as psS,
                ):
                  for j in range(NJ):
                    jsl = slice(j * 512, (j + 1) * 512)
                    for ti, (w_sb, wcol, dest) in enumerate((
                        (wq_sb, wqc_sb, qpre),
                        (wk_sb, wkc_sb, kT),
                    )):
                        ssq_ps = psS.tile([1, 512], F32, tag="ssq", name=f"ssq{j}{ti}")
                        for fc in range(FC):
                            fsl = slice(fc * 128, (fc + 1) * 128)
                            prj = psA.tile(
                                [128, 512], F32, tag="proj", name=f"prj{j}{ti}{fc}"
                            )
                            for kc in range(KC):
                                nc.tensor.matmul(
                                    prj[:],
                                    w_sb[:, kc, fsl],
                                    xsb[:, kc, jsl],
                                    start=(kc == 0),
                                    stop=(kc == KC - 1),
                                )
                            qw = pwa.tile([128, 512], BF16, tag="qw", name=f"qw{j}{ti}{fc}")
                            nc.scalar.activation(qw[:], prj[:], AF.Copy)
                            # sum-of-squares partial (DVE: only one PSUM input)
                            q2 = pwa.tile([128, 512], BF16, tag="q2", name=f"q2_{j}{ti}{fc}")
                            nc.vector.tensor_mul(q2[:], prj[:], qw[:])
                            nc.tensor.matmul(
                                ssq_ps[:],
                                ones_col_bf[:],
                                q2[:],
                                start=(fc == 0),
                                stop=(fc == FC - 1),
                            )
                            # rope with norm weight folded in
                            tcos = pwa.tile([128, 512], BF16, tag="tcos", name=f"tc{j}{ti}{fc}")
                            nc.vector.scalar_tensor_tensor(
                                tcos[:], qw[:], wcol[:, fc : fc + 1], cos_sb[:, jsl],
                                op0=mul, op1=mul,
                            )
                            tsin = pwa.tile([128, 512], BF16, tag="tsin", name=f"ts{j}{ti}{fc}")
                            nc.vector.scalar_tensor_tensor(
                                tsin[:], qw[:], wcol[:, fc : fc + 1], sin_sb[:, jsl],
                                op0=mul, op1=mul,
                            )
                            swp = psSw.tile([128, 512], F32, tag="swp", name=f"sw{j}{ti}{fc}")
                            nc.tensor.matmul(swp[:], psw[:], tsin[:], start=True, stop=True)
                            nc.vector.tensor_add(dest[:, fc, jsl], tcos[:], swp[:])
                        nc.scalar.activation(
                            ssqp[32 * ti : 32 * ti + 1, jsl], ssq_ps[:], AF.Copy
                        )

                # ---- RMSNorm allreduce (overlaps v projections below) -----
                nc.sync.dma_start(out=ssq_in[0:1, :], in_=ssqp[0:1, :])
                nc.sync.dma_start(out=ssq_in[1:2, :], in_=ssqp[32:33, :])
                nc.gpsimd.collective_compute(
                    "AllReduce",
                    mybir.AluOpType.add,
                    replica_groups=GROUPS4,
                    ins=[ssq_in[:]],
                    outs=[ssq_out[:]],
                )
                nc.sync.dma_start(out=sq2[0:1, :], in_=ssq_out[0:1, :])
                nc.sync.dma_start(out=sq2[32:33, :], in_=ssq_out[1:2, :])

                # rq = rsqrt(ssq/D + eps) = exp(-0.5*ln(ssq/D + eps))
                # rk8 = rq_k / 8      (score scale folded in, bias = -ln 8)
                nc.scalar.activation(
                    lnv[:], sq2[:], AF.Ln, scale=1.0 / D, bias=eps_t[:]
                )
                nc.scalar.activation(
                    rinv[:], lnv[:], AF.Exp, scale=-0.5, bias=bexp_t[:]
                )

                # ---- v projections (overlap the AllReduce) ----------------
                with tc.tile_pool(name="psV", bufs=4, space="PSUM") as psV:
                    for nt in range(NT):
                        nsl = slice((nt % 8) * 128, (nt % 8 + 1) * 128)
                        jsl = slice((nt // 8) * 1024, (nt // 8) * 1024 + 1024)
                        vps = psV.tile([128, FPC], F32, tag="v", name=f"v{nt}")
                        for kc in range(KC):
                            nc.tensor.matmul(
                                vps[:],
                                xsb[:, kc, jsl][:, nsl],
                                wv_sb[:, kc, :],
                                start=(kc == 0),
                                stop=(kc == KC - 1),
                            )
                        nc.scalar.activation(
                            v_aug[:, nt, :, 0:DH],
                            vps[:].rearrange("p (h f) -> p h f", f=DH),
                            AF.Copy,
                        )

                    # k-scales into column form [128, 16] via 16 rank-1 matmuls
                    rk_ps = psV.tile([128, NT], F32, tag="v", name="rkps")
                    for t in range(NT):
                        nc.tensor.matmul(
                            rk_ps[:, t : t + 1],
                            rinv[32:33, t * 128 : (t + 1) * 128],
                            one1_f32[32:33, :],
                            start=True,
                            stop=True,
                        )
                    nc.vector.tensor_copy(rmsk_col[:], rk_ps[:])

                # qn = qpre * broadcast(rq)
                nc.gpsimd.partition_broadcast(bq[:], rinv[0:1, :])
                for fc in range(FC):
                    nc.vector.tensor_mul(qn[:, fc, :], qpre[:, fc, :], bq[:])

            # ============== PHASE B: attention, per head ===================
            with (
                tc.tile_pool(name="pwb", bufs=3) as pwb,
                tc.tile_pool(name="ppb", bufs=3) as ppb,
                tc.tile_pool(name="psB", bufs=2, space="PSUM") as psB,
                tc.tile_pool(name="psOV", bufs=2, space="PSUM") as psOV,
            ):
                for h in range(HPC):
                    ch = h // 2
                    po = 64 * (h % 2)
                    ov = [
                        psOV.tile([DH + 1, 1024], F32, tag="ov", name=f"ov{h}_{i}")
                        for i in range(NJ2)
                    ]
                    units = [(mt, hf) for mt in range(NT) for hf in range(NJ2)]

                    def emit_scores(k):
                        mt, hf = units[k]
                        s_ps = psB.tile(
                            [128, 1024], F32, tag="S", name=f"S{h}_{mt}_{hf}"
                        )
                        # col-packed pairs: m-halves go to array col groups
                        # 0/64 concurrently (tile_position auto-derived)
                        for j2 in range(2):
                            jsl = slice(
                                hf * 1024 + j2 * 512, hf * 1024 + j2 * 512 + 512
                            )
                            for mh in range(2):
                                msl = slice(
                                    mt * 128 + mh * 64, mt * 128 + mh * 64 + 64
                                )
                                nc.tensor.matmul(
                                    s_ps[mh * 64 : (mh + 1) * 64,
                                         j2 * 512 : (j2 + 1) * 512],
                                    kT[po : po + 64, ch, msl],
                                    qn[po : po + 64, ch, jsl],
                                    start=True,
                                    stop=True,
                                )
                        return s_ps

                    def emit_expov(k, s_ps):
                        mt, hf = units[k]
                        p_t = ppb.tile(
                            [128, 1024], BF16, tag="P", name=f"P{h}_{mt}_{hf}"
                        )
                        nc.scalar.activation(
                            p_t[:], s_ps[:], AF.Exp,
                            scale=rmsk_col[:, mt : mt + 1],
                        )
                        if dbg and h == 0 and mt == 0 and hf == 0:
                            nc.sync.dma_start(out=dbg_pt[:], in_=p_t[:])
                        for j2 in range(2):
                            nc.tensor.matmul(
                                ov[hf][:, j2 * 512 : (j2 + 1) * 512],
                                v_aug[:, mt, h, :],
                                p_t[:, j2 * 512 : (j2 + 1) * 512],
                                start=(mt == 0),
                                stop=(mt == NT - 1),
                            )

                    # software pipeline: scores(k+1) is emitted before ov(k)
                    # so the in-order PE queue never stalls on the exp
                    prev = None
                    for k in range(len(units)):
                        cur = emit_scores(k)
                        if prev is not None:
                            emit_expov(k - 1, prev)
                        prev = cur
                    emit_expov(len(units) - 1, prev)

                    for hf in range(NJ2):
                        jsl = slice(hf * 1024, (hf + 1) * 1024)
                        # rec = 1/den via ACT ln->exp (same table set as the
                        # softmax exp, reads the PSUM denominator row directly)
                        dln = pwb.tile([65, 1024], F32, tag="dln", name=f"dl{h}{hf}")
                        nc.scalar.activation(
                            dln[DH : DH + 1, :], ov[hf][DH : DH + 1, :], AF.Ln
                        )
                        rec = pwb.tile([65, 1024], F32, tag="rec", name=f"rec{h}{hf}")
                        nc.scalar.activation(
                            rec[DH : DH + 1, :], dln[DH : DH + 1, :], AF.Exp,
                            scale=-1.0,
                        )
                        rec0 = pwb.tile([1, 1024], F32, tag="rec0", name=f"r0{h}{hf}")
                        nc.sync.dma_start(out=rec0[:], in_=rec[DH : DH + 1, :])
                        if dbg:
                            nc.sync.dma_start(
                                out=dbg_rec[2 * h + hf : 2 * h + hf + 1, :], in_=rec0[:]
                            )
                        bcast = pwb.tile([DH, 1024], F32, tag="bcast", name=f"bc{h}{hf}")
                        nc.gpsimd.partition_broadcast(bcast[:], rec0[:])
                        nc.vector.tensor_mul(
                            o_sb[po : po + 64, ch, jsl], ov[hf][0:DH, :], bcast[:]
                        )
                    if h % 2 == 1:
                        nc.sync.dma_start(out=ag_in[ch][:], in_=o_sb[:, ch, :])
                        nc.gpsimd.collective_compute(
                            "AllGather",
                            mybir.AluOpType.bypass,
                            replica_groups=GROUP8,
                            ins=[ag_in[ch][:]],
                            outs=[ag_out[ch][:]],
                        )

                # og gathers go AFTER the whole head loop: their AG-completion
                # waits must not sit in the gpsimd queue ahead of the per-head
                # partition_broadcast ops (that stalls ov-PSUM recycling)
                for ch in range(FC):
                    for kc in range(ch, KC, FC):
                        nc.gpsimd.indirect_dma_start(
                            out=og[:, kc, :],
                            out_offset=None,
                            in_=ag_out[ch][:].rearrange("r (j n) -> (r j) n", n=512),
                            in_offset=bass.IndirectOffsetOnAxis(
                                ap=gidx_sb[:, kc : kc + 1], axis=0
                            ),
                        )

            if dbg:
                nc.gpsimd.dma_start(out=dbg_qn[:], in_=qn[:])
                nc.gpsimd.dma_start(out=dbg_kt[:], in_=kT[:])
                nc.sync.dma_start(out=dbg_ri[:], in_=rinv[:])
                nc.sync.dma_start(out=dbg_rm[:], in_=rmsk_col[:])
                nc.sync.dma_start(out=dbg_va[:], in_=v_aug[:])
                nc.sync.dma_start(out=dbg_osb[:], in_=o_sb[:])
                nc.sync.dma_start(out=dbg_og[:], in_=og[:])

            # ========= PHASE C: output projection ==========================
            # even (ch 0) contractions first: they are ready while the ch 1
            # AllGather is still in flight
            kc_order = list(range(0, KC, 2)) + list(range(1, KC, 2))
            with (
                tc.tile_pool(name="pc", bufs=2) as pc,
                tc.tile_pool(name="psC", bufs=2, space="PSUM") as psC,
            ):
                for ntl in range(4):
                    yps = psC.tile([128, D], F32, tag="y", name=f"y{ntl}")
                    for dc in range(2):
                        dsl = slice(dc * 512, (dc + 1) * 512)
                        for i, kc in enumerate(kc_order):
                            nc.tensor.matmul(
                                yps[:, dsl],
                                og[:, kc, ntl * 128 : (ntl + 1) * 128],
                                wo_sb[:, kc, dsl],
                                start=(i == 0),
                                stop=False,
                            )
                        nc.tensor.matmul(
                            yps[:, dsl],
                            ones_row_bf[:],
                            bo_sb[:, dsl],
                            start=False,
                            stop=True,
                        )
                    ysb = pc.tile([128, D], F32, tag="ysb", name=f"ysb{ntl}")
                    nc.vector.tensor_copy(ysb[:], yps[:])
                    nc.sync.dma_start(
                        out=out_d[ntl * 128 : (ntl + 1) * 128, :], in_=ysb[:]
                    )

    nc.compile()
    return nc


def _rope_tables():
    """cos/sin tables matching the reference's f32 angle computation.

    C[d, n] = cos(n * theta[d//2]);  Ssw[2i] = +sin, Ssw[2i+1] = -sin
    (Ssw is the swapped-operand multiplier: rope = x*C + swap(x*Ssw)).
    Tiled x2 along partitions to cover a 2-head (128-row) chunk.
    """
    i2 = np.arange(0, DH, 2, dtype=np.float32)
    theta = (1.0 / (10000.0 ** (i2 / DH))).astype(np.float32)  # (32,)
    ang = np.arange(N, dtype=np.float32)[:, None] * theta[None, :]  # (N, 32) f32
    cos = np.cos(ang.astype(np.float64)).astype(np.float32).T  # (32, N)
    sin = np.sin(ang.astype(np.float64)).astype(np.float32).T
    cos_d = np.repeat(cos, 2, axis=0)  # (64, N)
    ssw = np.repeat(sin, 2, axis=0)
    ssw[1::2, :] *= -1.0
    cos_t = np.tile(cos_d, (2, 1)).astype(np.float32)  # (128, N)
    sin_t = np.tile(ssw, (2, 1)).astype(np.float32)
    return cos_t, sin_t


def _rearr(w):
    # [D, F] -> [128, KC, F] grouping the contraction dim into 128-row chunks
    d, f = w.shape
    return np.ascontiguousarray(
        w.reshape(KC, 128, f).transpose(1, 0, 2).astype(BF)
    )


def kernel(x, Wq, Wkv, norm_q_w, norm_k_w, Wo, bo, _trace=False, _dbg=False):
    global _CACHED_NC
    x = np.asarray(x, dtype=np.float32)
    Wq = np.asarray(Wq, dtype=np.float32)
    Wkv = np.asarray(Wkv, dtype=np.float32)
    norm_q_w = np.asarray(norm_q_w, dtype=np.float32)
    norm_k_w = np.asarray(norm_k_w, dtype=np.float32)
    Wo = np.asarray(Wo, dtype=np.float32)
    bo = np.asarray(bo, dtype=np.float32)

    cos_t, sin_t = _rope_tables()
    if _dbg:
        nc = build(dbg=True)
    else:
        if _CACHED_NC is None:
            _CACHED_NC = build()
        nc = _CACHED_NC

    in_maps = []
    for c in range(CORES):
        b, g = c // 4, c % 4
        fsl = slice(g * FPC, (g + 1) * FPC)
        gidx = np.empty((128, KC), dtype=np.int32)
        p = np.arange(128)
        for kc in range(KC):
            gidx[:, kc] = (128 * (4 * b + kc // FC) + p) * NJ + g
        in_maps.append(
            {
                "xT": _rearr(np.ascontiguousarray(x[b].T)),
                "wq": _rearr(Wq[:, fsl]),
                "wk": _rearr(Wkv[:, fsl]),
                "wv": _rearr(Wkv[:, D + g * FPC : D + (g + 1) * FPC]),
                "wo": _rearr(Wo),
                "bo": bo.reshape(1, D).astype(BF),
                "wqc": np.ascontiguousarray(
                    norm_q_w[fsl].reshape(FC, 128).T
                ).astype(BF),
                "wkc": np.ascontiguousarray(
                    norm_k_w[fsl].reshape(FC, 128).T
                ).astype(BF),
                "cos_t": cos_t.astype(BF),
                "sin_t": sin_t.astype(BF),
                "gidx": gidx,
            }
        )

    res = run_bass_kernel_spmd(nc, in_maps, list(range(CORES)), trace=_trace)
    out = np.empty((B, N, D), dtype=np.float32)
    for c in range(CORES):
        b, g = c // 4, c % 4
        out[b, g * 512 : (g + 1) * 512, :] = res.results[c]["out"]
    if _trace or _dbg:
        return out, res
    return out


# revision 31
# speedup vs baseline: 1.4645x; 1.0538x over previous
"""Distributed Bass kernel for nn_Attention (B=2, N=2048, D=1024, H=16, DH=64) on 8 trn2 cores.

Sharding: data-parallel over batch (cores 0-3 -> b=0, 4-7 -> b=1), tensor-parallel
over heads (4 heads / 256 inner features per core).  v2 design (bf16 everywhere):
  all matmuls bf16 (weights/x pre-cast+rearranged on host, fp32 PSUM accumulate),
  q/k projections + rope first, ssq AllReduce overlapped with v projections,
  RMSNorm scales via ACT ln/exp (single activation table set for whole kernel),
  attention scores col-packed 2x via tile_position quadrants (DH=64),
  softmax denominator via ones-row in v, fast-approx reciprocal,
  8-core AllGather per 2-head chunk (bf16), out-projection ordered so the
  second AllGather overlaps the first half of the output matmuls.
Host assembles the (2, 2048, 1024) output from the 8 (512, 1024) shards.
"""
import os
import sys

for _p in ("/opt/trn_rl_repo", "/root/.axon_site/_ro/trn_rl_repo"):
    if os.path.isdir(_p) and _p not in sys.path:
        sys.path.insert(0, _p)

import numpy as np
import ml_dtypes
import concourse.bass as bass
import concourse.mybir as mybir
import concourse.tile as tile
from concourse import bacc
from concourse.bass_utils import run_bass_kernel_spmd

dt = mybir.dt
AF = mybir.ActivationFunctionType
F32, BF16, I32 = dt.float32, dt.bfloat16, dt.int32
BF = ml_dtypes.bfloat16

B, N, D = 2, 2048, 1024
H, DH = 16, 64
HPC = 4            # heads per core
FPC = HPC * DH     # 256 inner features per core
KC = D // 128      # 8 contraction chunks
FC = FPC // 128    # 2 feature chunks per core
NJ = N // 512      # 4 gather chunks (AllGather slice granularity)
NT = N // 128      # 16 m-tiles
NJ2 = N // 1024    # 2 compute chunks
EPS = 1e-6
CORES = 8
GROUPS4 = [[0, 1, 2, 3], [4, 5, 6, 7]]
GROUP8 = [list(range(CORES))]

_CACHED_NC = None


def build(dbg=False):
    nc = bacc.Bacc("TRN2", target_bir_lowering=False, debug=False, num_devices=CORES)

    xT = nc.dram_tensor("xT", [128, KC, N], BF16, kind="ExternalInput")
    wq_d = nc.dram_tensor("wq", [128, KC, FPC], BF16, kind="ExternalInput")
    wk_d = nc.dram_tensor("wk", [128, KC, FPC], BF16, kind="ExternalInput")
    wv_d = nc.dram_tensor("wv", [128, KC, FPC], BF16, kind="ExternalInput")
    wo_d = nc.dram_tensor("wo", [128, KC, D], BF16, kind="ExternalInput")
    bo_d = nc.dram_tensor("bo", [1, D], BF16, kind="ExternalInput")
    wqc_d = nc.dram_tensor("wqc", [128, FC], BF16, kind="ExternalInput")
    wkc_d = nc.dram_tensor("wkc", [128, FC], BF16, kind="ExternalInput")
    cos_d = nc.dram_tensor("cos_t", [128, N], BF16, kind="ExternalInput")
    sin_d = nc.dram_tensor("sin_t", [128, N], BF16, kind="ExternalInput")
    gidx_d = nc.dram_tensor("gidx", [128, KC], I32, kind="ExternalInput")
    out_d = nc.dram_tensor("out", [512, D], F32, kind="ExternalOutput")

    if dbg:
        dbg_qn = nc.dram_tensor("dbg_qn", [128, FC, N], BF16, kind="ExternalOutput")
        dbg_kt = nc.dram_tensor("dbg_kt", [128, FC, N], BF16, kind="ExternalOutput")
        dbg_ri = nc.dram_tensor("dbg_ri", [33, N], F32, kind="ExternalOutput")
        dbg_rm = nc.dram_tensor("dbg_rm", [128, NT], F32, kind="ExternalOutput")
        dbg_va = nc.dram_tensor(
            "dbg_va", [128, NT, HPC, DH + 1], BF16, kind="ExternalOutput"
        )
        dbg_osb = nc.dram_tensor("dbg_osb", [128, FC, N], BF16, kind="ExternalOutput")
        dbg_og = nc.dram_tensor("dbg_og", [128, KC, 512], BF16, kind="ExternalOutput")
        dbg_pt = nc.dram_tensor("dbg_pt", [128, 1024], BF16, kind="ExternalOutput")
        dbg_rec = nc.dram_tensor("dbg_rec", [HPC * 2, 1024], F32, kind="ExternalOutput")

    # collective bounce buffers
    ssq_in = nc.dram_tensor("ssq_in", [2, N], F32)
    ssq_out = nc.dram_tensor("ssq_out", [2, N], F32)
    ag_in = [nc.dram_tensor(f"ag_in{c}", [128, N], BF16) for c in range(FC)]
    ag_out = [
        nc.dram_tensor(f"ag_out{c}", [CORES * 128, N], BF16, addr_space="Shared")
        for c in range(FC)
    ]

    with tile.TileContext(nc) as tc:
        with tc.tile_pool(name="persist", bufs=1) as pp:
            # ---- constants ------------------------------------------------
            ones_col32 = pp.tile([128, 1], F32, tag="onesc32")
            nc.gpsimd.memset(ones_col32[:], 1.0)
            ones_col_bf = pp.tile([128, 1], BF16, tag="onescbf")
            nc.vector.tensor_copy(ones_col_bf[:], ones_col32[:])
            ones_row32 = pp.tile([1, 128], F32, tag="onesr32")
            nc.gpsimd.memset(ones_row32[:], 1.0)
            ones_row_bf = pp.tile([1, 128], BF16, tag="onesrbf")
            nc.vector.tensor_copy(ones_row_bf[:], ones_row32[:])
            one1_f32 = pp.tile([33, 1], F32, tag="one1")
            nc.gpsimd.memset(one1_f32[:], 1.0)
            # activation bias values at consumer base partitions
            eps_t = pp.tile([33, 1], F32, tag="eps")
            nc.gpsimd.memset(eps_t[:], EPS)
            bexp_t = pp.tile([33, 1], F32, tag="bexp")
            nc.gpsimd.memset(bexp_t[:], 0.0)
            nc.gpsimd.memset(bexp_t[32:33, :], -float(np.log(8.0)))

            wqc_sb = pp.tile([128, FC], BF16, tag="wqc")
            wkc_sb = pp.tile([128, FC], BF16, tag="wkc")
            nc.sync.dma_start(out=wqc_sb[:], in_=wqc_d[:])
            nc.sync.dma_start(out=wkc_sb[:], in_=wkc_d[:])
            gidx_sb = pp.tile([128, KC], I32, tag="gidx")
            nc.sync.dma_start(out=gidx_sb[:], in_=gidx_d[:])

            # ---- big persistent tensors ----------------------------------
            # DMA order matters: the first q/k matmul group needs xsb j=0 and
            # wq only, so those go first on the queue
            xsb = pp.tile([128, KC, N], BF16, tag="xsb")
            wq_sb = pp.tile([128, KC, FPC], BF16, tag="wq")
            wk_sb = pp.tile([128, KC, FPC], BF16, tag="wk")
            wv_sb = pp.tile([128, KC, FPC], BF16, tag="wv")
            cos_sb = pp.tile([128, N], BF16, tag="cos")
            sin_sb = pp.tile([128, N], BF16, tag="sin")
            nc.sync.dma_start(out=xsb[:, :, 0:512], in_=xT[:, :, 0:512])
            nc.sync.dma_start(out=wq_sb[:], in_=wq_d[:])
            nc.sync.dma_start(out=cos_sb[:, 0:512], in_=cos_d[:, 0:512])
            nc.sync.dma_start(out=sin_sb[:, 0:512], in_=sin_d[:, 0:512])
            nc.sync.dma_start(out=wk_sb[:], in_=wk_d[:])
            for j in range(1, NJ):
                jsl = slice(j * 512, (j + 1) * 512)
                nc.sync.dma_start(out=xsb[:, :, jsl], in_=xT[:, :, jsl])
                nc.sync.dma_start(out=cos_sb[:, jsl], in_=cos_d[:, jsl])
                nc.sync.dma_start(out=sin_sb[:, jsl], in_=sin_d[:, jsl])
            nc.sync.dma_start(out=wv_sb[:], in_=wv_d[:])
            wo_sb = pp.tile([128, KC, D], BF16, tag="wo")
            nc.gpsimd.dma_start(out=wo_sb[:], in_=wo_d[:])
            bo_sb = pp.tile([1, D], BF16, tag="bo")
            nc.gpsimd.dma_start(out=bo_sb[:], in_=bo_d[:])

            kT = pp.tile([128, FC, N], BF16, tag="kT")
            qn = pp.tile([128, FC, N], BF16, tag="qn")
            v_aug = pp.tile([128, NT, HPC, DH + 1], BF16, tag="vaug")
            nc.vector.tensor_copy(
                v_aug[:, :, :, DH : DH + 1],
                ones_col32[:].to_broadcast([128, NT, HPC, 1]),
            )
            o_sb = pp.tile([128, FC, N], BF16, tag="osb")
            rmsk_col = pp.tile([128, NT], F32, tag="rmskcol")
            og = pp.tile([128, KC, 512], BF16, tag="og")

            # ================= PHASE A: q/k projections + rope =============
            with (
                tc.tile_pool(name="pa", bufs=1) as pa,
                tc.tile_pool(name="pwa", bufs=4) as pwa,
            ):
                # pair-swap permutation matrix: psw[p, 2f+e] = 1 iff p == 2f+1-e
                psw32 = pa.tile([128, 128], F32, tag="psw32")
                nc.gpsimd.memset(psw32[:], 0.0)
                nc.gpsimd.affine_select(
                    out=psw32[:].rearrange("p (f e) -> p f e", e=2),
                    in_=psw32[:].rearrange("p (f e) -> p f e", e=2),
                    compare_op=mybir.AluOpType.not_equal,
                    fill=1.0,
                    base=-1,
                    pattern=[[-2, 64], [1, 2]],
                    channel_multiplier=1,
                )
                psw = pa.tile([128, 128], BF16, tag="psw")
                nc.vector.tensor_copy(psw[:], psw32[:])

                qpre = pa.tile([128, FC, N], BF16, tag="qpre")
                # row-vector stripes at base partitions 0 (q) and 32 (k)
                ssqp = pa.tile([33, N], F32, tag="ssqp")
                sq2 = pa.tile([33, N], F32, tag="sq2")
                lnv = pa.tile([33, N], F32, tag="lnv")
                rinv = pp.tile([33, N], F32, tag="rinv")
                bq = pa.tile([128, N], F32, tag="bq")
                # rows 1-31 are never written by the ssq path but are read by
                # the combined [33, N] ln/exp below; keep them finite
                nc.gpsimd.memset(sq2[:], 1.0)

                mul = mybir.AluOpType.mult
                with (
                    tc.tile_pool(name="psA", bufs=3, space="PSUM") as psA,
                    tc.tile_pool(name="psSw", bufs=2, space="PSUM") as psSw,
                    tc.tile_pool(name="psS", bufs=2, space="PSUM") as psS,
                ):
                  for j in range(NJ):
                    jsl = slice(j * 512, (j + 1) * 512)
                    for ti, (w_sb, wcol, dest) in enumerate((
                        (wq_sb, wqc_sb, qpre),
                        (wk_sb, wkc_sb, kT),
                    )):
                        ssq_ps = psS.tile([1, 512], F32, tag="ssq", name=f"ssq{j}{ti}")
                        for fc in range(FC):
                            fsl = slice(fc * 128, (fc + 1) * 128)
                            prj = psA.tile(
                                [128, 512], F32, tag="proj", name=f"prj{j}{ti}{fc}"
                            )
                            for kc in range(KC):
                                nc.tensor.matmul(
                                    prj[:],
                                    w_sb[:, kc, fsl],
                                    xsb[:, kc, jsl],
                                    start=(kc == 0),
                                    stop=(kc == KC - 1),
                                )
                            qw = pwa.tile([128, 512], BF16, tag="qw", name=f"qw{j}{ti}{fc}")
                            nc.scalar.activation(qw[:], prj[:], AF.Copy)
                            # sum-of-squares partial (DVE: only one PSUM input)
                            q2 = pwa.tile([128, 512], BF16, tag="q2", name=f"q2_{j}{ti}{fc}")
                            nc.vector.tensor_mul(q2[:], prj[:], qw[:])
                            nc.tensor.matmul(
                                ssq_ps[:],
                                ones_col_bf[:],
                                q2[:],
                                start=(fc == 0),
                                stop=(fc == FC - 1),
                            )
                            # rope with norm weight folded in
                            tcos = pwa.tile([128, 512], BF16, tag="tcos", name=f"tc{j}{ti}{fc}")
                            nc.vector.scalar_tensor_tensor(
                                tcos[:], qw[:], wcol[:, fc : fc + 1], cos_sb[:, jsl],
                                op0=mul, op1=mul,
                            )
                            tsin = pwa.tile([128, 512], BF16, tag="tsin", name=f"ts{j}{ti}{fc}")
                            nc.vector.scalar_tensor_tensor(
                                tsin[:], qw[:], wcol[:, fc : fc + 1], sin_sb[:, jsl],
                                op0=mul, op1=mul,
                            )
                            swp = psSw.tile([128, 512], F32, tag="swp", name=f"sw{j}{ti}{fc}")
                            nc.tensor.matmul(swp[:], psw[:], tsin[:], start=True, stop=True)
                            nc.vector.tensor_add(dest[:, fc, jsl], tcos[:], swp[:])
                        nc.scalar.activation(
                            ssqp[32 * ti : 32 * ti + 1, jsl], ssq_ps[:], AF.Copy
                        )

                # ---- RMSNorm allreduce (overlaps v projections below) -----
                nc.sync.dma_start(out=ssq_in[0:1, :], in_=ssqp[0:1, :])
                nc.sync.dma_start(out=ssq_in[1:2, :], in_=ssqp[32:33, :])
                nc.gpsimd.collective_compute(
                    "AllReduce",
                    mybir.AluOpType.add,
                    replica_groups=GROUPS4,
                    ins=[ssq_in[:]],
                    outs=[ssq_out[:]],
                )
                nc.sync.dma_start(out=sq2[0:1, :], in_=ssq_out[0:1, :])
                nc.sync.dma_start(out=sq2[32:33, :], in_=ssq_out[1:2, :])

                # rq = rsqrt(ssq/D + eps) = exp(-0.5*ln(ssq/D + eps))
                # rk8 = rq_k / 8      (score scale folded in, bias = -ln 8)
                nc.scalar.activation(
                    lnv[:], sq2[:], AF.Ln, scale=1.0 / D, bias=eps_t[:]
                )
                nc.scalar.activation(
                    rinv[:], lnv[:], AF.Exp, scale=-0.5, bias=bexp_t[:]
                )

                # ---- v projections (overlap the AllReduce) ----------------
                with tc.tile_pool(name="psV", bufs=4, space="PSUM") as psV:
                    for nt in range(NT):
                        nsl = slice((nt % 8) * 128, (nt % 8 + 1) * 128)
                        jsl = slice((nt // 8) * 1024, (nt // 8) * 1024 + 1024)
                        vps = psV.tile([128, FPC], F32, tag="v", name=f"v{nt}")
                        for kc in range(KC):
                            nc.tensor.matmul(
                                vps[:],
                                xsb[:, kc, jsl][:, nsl],
                                wv_sb[:, kc, :],
                                start=(kc == 0),
                                stop=(kc == KC - 1),
                            )
                        nc.scalar.activation(
                            v_aug[:, nt, :, 0:DH],
                            vps[:].rearrange("p (h f) -> p h f", f=DH),
                            AF.Copy,
                        )

                    # k-scales into column form [128, 16] via 16 rank-1 matmuls
                    rk_ps = psV.tile([128, NT], F32, tag="v", name="rkps")
                    for t in range(NT):
                        nc.tensor.matmul(
                            rk_ps[:, t : t + 1],
                            rinv[32:33, t * 128 : (t + 1) * 128],
                            one1_f32[32:33, :],
                            start=True,
                            stop=True,
                        )
                    nc.vector.tensor_copy(rmsk_col[:], rk_ps[:])

                # qn = qpre * broadcast(rq)
                nc.gpsimd.partition_broadcast(bq[:], rinv[0:1, :])
                for fc in range(FC):
                    nc.vector.tensor_mul(qn[:, fc, :], qpre[:, fc, :], bq[:])

            # ============== PHASE B: attention, per head ===================
            with (
                tc.tile_pool(name="pwb", bufs=4) as pwb,
                tc.tile_pool(name="ppb", bufs=3) as ppb,
                tc.tile_pool(name="psB", bufs=2, space="PSUM") as psB,
                tc.tile_pool(name="psOV", bufs=2, space="PSUM") as psOV,
            ):
                for h in range(HPC):
                    ch = h // 2
                    po = 64 * (h % 2)
                    ov = [
                        psOV.tile([DH + 1, 1024], F32, tag="ov", name=f"ov{h}_{i}")
                        for i in range(NJ2)
                    ]
                    units = [(mt, hf) for mt in range(NT) for hf in range(NJ2)]

                    def emit_scores(k):
                        mt, hf = units[k]
                        s_ps = psB.tile(
                            [128, 1024], F32, tag="S", name=f"S{h}_{mt}_{hf}"
                        )
                        # col-packed pairs: m-halves go to array col groups
                        # 0/64 concurrently (tile_position auto-derived)
                        for j2 in range(2):
                            jsl = slice(
                                hf * 1024 + j2 * 512, hf * 1024 + j2 * 512 + 512
                            )
                            for mh in range(2):
                                msl = slice(
                                    mt * 128 + mh * 64, mt * 128 + mh * 64 + 64
                                )
                                nc.tensor.matmul(
                                    s_ps[mh * 64 : (mh + 1) * 64,
                                         j2 * 512 : (j2 + 1) * 512],
                                    kT[po : po + 64, ch, msl],
                                    qn[po : po + 64, ch, jsl],
                                    start=True,
                                    stop=True,
                                )
                        return s_ps

                    def emit_expov(k, s_ps):
                        mt, hf = units[k]
                        p_t = ppb.tile(
                            [128, 1024], BF16, tag="P", name=f"P{h}_{mt}_{hf}"
                        )
                        nc.scalar.activation(
                            p_t[:], s_ps[:], AF.Exp,
                            scale=rmsk_col[:, mt : mt + 1],
                        )
                        if dbg and h == 0 and mt == 0 and hf == 0:
                            nc.sync.dma_start(out=dbg_pt[:], in_=p_t[:])
                        for j2 in range(2):
                            nc.tensor.matmul(
                                ov[hf][:, j2 * 512 : (j2 + 1) * 512],
                                v_aug[:, mt, h, :],
                                p_t[:, j2 * 512 : (j2 + 1) * 512],
                                start=(mt == 0),
                                stop=(mt == NT - 1),
                            )

                    # software pipeline: scores(k+1) is emitted before ov(k)
                    # so the in-order PE queue never stalls on the exp
                    prev = None
                    for k in range(len(units)):
                        cur = emit_scores(k)
                        if prev is not None:
                            emit_expov(k - 1, prev)
                        prev = cur
                    emit_expov(len(units) - 1, prev)

                    for hf in range(NJ2):
                        jsl = slice(hf * 1024, (hf + 1) * 1024)
                        # denominator row: psum partition 64 -> sbuf partition 0
                        # (DVE-only tail keeps the ACT queue free for exps)
                        den0 = pwb.tile([1, 1024], F32, tag="den0", name=f"d0{h}{hf}")
                        nc.vector.tensor_copy(den0[:], ov[hf][DH : DH + 1, :])
                        rec0 = pwb.tile([1, 1024], F32, tag="rec0", name=f"r0{h}{hf}")
                        nc.vector.reciprocal_approx_fast(rec0[:], den0[:])
                        if dbg:
                            nc.sync.dma_start(
                                out=dbg_rec[2 * h + hf : 2 * h + hf + 1, :], in_=rec0[:]
                            )
                        bcast = pwb.tile([DH, 1024], F32, tag="bcast", name=f"bc{h}{hf}")
                        nc.gpsimd.partition_broadcast(bcast[:], rec0[:])
                        nc.vector.tensor_mul(
                            o_sb[po : po + 64, ch, jsl], ov[hf][0:DH, :], bcast[:]
                        )
                    if h == 2:
                        # ch0 gather slices (AG0 landed during h2) so the phase
                        # C even-kc matmuls can run while AG1 is in flight
                        for kc in range(0, KC, FC):
                            nc.gpsimd.indirect_dma_start(
                                out=og[:, kc, :],
                                out_offset=None,
                                in_=ag_out[0][:].rearrange("r (j n) -> (r j) n", n=512),
                                in_offset=bass.IndirectOffsetOnAxis(
                                    ap=gidx_sb[:, kc : kc + 1], axis=0
                                ),
                            )
                    if h % 2 == 1:
                        nc.sync.dma_start(out=ag_in[ch][:], in_=o_sb[:, ch, :])
                        nc.gpsimd.collective_compute(
                            "AllGather",
                            mybir.AluOpType.bypass,
                            replica_groups=GROUP8,
                            ins=[ag_in[ch][:]],
                            outs=[ag_out[ch][:]],
                        )

                # ch1 gathers wait on AG1 (the gpsimd queue is done with all
                # per-head work by now, so the wait blocks nothing)
                for kc in range(1, KC, FC):
                    nc.gpsimd.indirect_dma_start(
                        out=og[:, kc, :],
                        out_offset=None,
                        in_=ag_out[1][:].rearrange("r (j n) -> (r j) n", n=512),
                        in_offset=bass.IndirectOffsetOnAxis(
                            ap=gidx_sb[:, kc : kc + 1], axis=0
                        ),
                    )

            if dbg:
                nc.gpsimd.dma_start(out=dbg_qn[:], in_=qn[:])
                nc.gpsimd.dma_start(out=dbg_kt[:], in_=kT[:])
                nc.sync.dma_start(out=dbg_ri[:], in_=rinv[:])
                nc.sync.dma_start(out=dbg_rm[:], in_=rmsk_col[:])
                nc.sync.dma_start(out=dbg_va[:], in_=v_aug[:])
                nc.sync.dma_start(out=dbg_osb[:], in_=o_sb[:])
                nc.sync.dma_start(out=dbg_og[:], in_=og[:])

            # ========= PHASE C: output projection ==========================
            # even (ch 0) contractions first: they are ready while the ch 1
            # AllGather is still in flight
            kc_order = list(range(0, KC, 2)) + list(range(1, KC, 2))
            with (
                tc.tile_pool(name="pc", bufs=2) as pc,
                tc.tile_pool(name="psC", bufs=2, space="PSUM") as psC,
            ):
                for ntl in range(4):
                    yps = psC.tile([128, D], F32, tag="y", name=f"y{ntl}")
                    for dc in range(2):
                        dsl = slice(dc * 512, (dc + 1) * 512)
                        for i, kc in enumerate(kc_order):
                            nc.tensor.matmul(
                                yps[:, dsl],
                                og[:, kc, ntl * 128 : (ntl + 1) * 128],
                                wo_sb[:, kc, dsl],
                                start=(i == 0),
                                stop=False,
                            )
                        nc.tensor.matmul(
                            yps[:, dsl],
                            ones_row_bf[:],
                            bo_sb[:, dsl],
                            start=False,
                            stop=True,
                        )
                    ysb = pc.tile([128, D], F32, tag="ysb", name=f"ysb{ntl}")
                    nc.vector.tensor_copy(ysb[:], yps[:])
                    nc.sync.dma_start(
                        out=out_d[ntl * 128 : (ntl + 1) * 128, :], in_=ysb[:]
                    )

    nc.compile()
    return nc


def _rope_tables():
    """cos/sin tables matching the reference's f32 angle computation.

    C[d, n] = cos(n * theta[d//2]);  Ssw[2i] = +sin, Ssw[2i+1] = -sin
    (Ssw is the swapped-operand multiplier: rope = x*C + swap(x*Ssw)).
    Tiled x2 along partitions to cover a 2-head (128-row) chunk.
    """
    i2 = np.arange(0, DH, 2, dtype=np.float32)
    theta = (1.0 / (10000.0 ** (i2 / DH))).astype(np.float32)  # (32,)
    ang = np.arange(N, dtype=np.float32)[:, None] * theta[None, :]  # (N, 32) f32
    cos = np.cos(ang.astype(np.float64)).astype(np.float32).T  # (32, N)
    sin = np.sin(ang.astype(np.float64)).astype(np.float32).T
    cos_d = np.repeat(cos, 2, axis=0)  # (64, N)
    ssw = np.repeat(sin, 2, axis=0)
    ssw[1::2, :] *= -1.0
    cos_t = np.tile(cos_d, (2, 1)).astype(np.float32)  # (128, N)
    sin_t = np.tile(ssw, (2, 1)).astype(np.float32)
    return cos_t, sin_t


def _rearr(w):
    # [D, F] -> [128, KC, F] grouping the contraction dim into 128-row chunks
    d, f = w.shape
    return np.ascontiguousarray(
        w.reshape(KC, 128, f).transpose(1, 0, 2).astype(BF)
    )


def kernel(x, Wq, Wkv, norm_q_w, norm_k_w, Wo, bo, _trace=False, _dbg=False):
    global _CACHED_NC
    x = np.asarray(x, dtype=np.float32)
    Wq = np.asarray(Wq, dtype=np.float32)
    Wkv = np.asarray(Wkv, dtype=np.float32)
    norm_q_w = np.asarray(norm_q_w, dtype=np.float32)
    norm_k_w = np.asarray(norm_k_w, dtype=np.float32)
    Wo = np.asarray(Wo, dtype=np.float32)
    bo = np.asarray(bo, dtype=np.float32)

    cos_t, sin_t = _rope_tables()
    if _dbg:
        nc = build(dbg=True)
    else:
        if _CACHED_NC is None:
            _CACHED_NC = build()
        nc = _CACHED_NC

    in_maps = []
    for c in range(CORES):
        b, g = c // 4, c % 4
        fsl = slice(g * FPC, (g + 1) * FPC)
        gidx = np.empty((128, KC), dtype=np.int32)
        p = np.arange(128)
        for kc in range(KC):
            gidx[:, kc] = (128 * (4 * b + kc // FC) + p) * NJ + g
        in_maps.append(
            {
                "xT": _rearr(np.ascontiguousarray(x[b].T)),
                "wq": _rearr(Wq[:, fsl]),
                "wk": _rearr(Wkv[:, fsl]),
                "wv": _rearr(Wkv[:, D + g * FPC : D + (g + 1) * FPC]),
                "wo": _rearr(Wo),
                "bo": bo.reshape(1, D).astype(BF),
                "wqc": np.ascontiguousarray(
                    norm_q_w[fsl].reshape(FC, 128).T
                ).astype(BF),
                "wkc": np.ascontiguousarray(
                    norm_k_w[fsl].reshape(FC, 128).T
                ).astype(BF),
                "cos_t": cos_t.astype(BF),
                "sin_t": sin_t.astype(BF),
                "gidx": gidx,
            }
        )

    res = run_bass_kernel_spmd(nc, in_maps, list(range(CORES)), trace=_trace)
    out = np.empty((B, N, D), dtype=np.float32)
    for c in range(CORES):
        b, g = c // 4, c % 4
        out[b, g * 512 : (g + 1) * 512, :] = res.results[c]["out"]
    if _trace or _dbg:
        return out, res
    return out


# revision 37
# speedup vs baseline: 1.5892x; 1.0852x over previous
"""Distributed Bass kernel for nn_Attention (B=2, N=2048, D=1024, H=16, DH=64) on 8 trn2 cores.

Sharding: data-parallel over batch (cores 0-3 -> b=0, 4-7 -> b=1), tensor-parallel
over heads (4 heads / 256 inner features per core).  v2 design (bf16 everywhere):
  all matmuls bf16 (weights/x pre-cast+rearranged on host, fp32 PSUM accumulate),
  q/k projections + rope first, ssq AllReduce overlapped with v projections,
  RMSNorm scales via ACT ln/exp (single activation table set for whole kernel),
  attention scores col-packed 2x via tile_position quadrants (DH=64),
  softmax denominator via ones-row in v, fast-approx reciprocal,
  8-core AllGather per 2-head chunk (bf16), out-projection ordered so the
  second AllGather overlaps the first half of the output matmuls.
Host assembles the (2, 2048, 1024) output from the 8 (512, 1024) shards.
"""
import os
import sys

for _p in ("/opt/trn_rl_repo", "/root/.axon_site/_ro/trn_rl_repo"):
    if os.path.isdir(_p) and _p not in sys.path:
        sys.path.insert(0, _p)

import numpy as np
import ml_dtypes
import concourse.bass as bass
import concourse.mybir as mybir
import concourse.tile as tile
from concourse import bacc
from concourse.bass_utils import run_bass_kernel_spmd

dt = mybir.dt
AF = mybir.ActivationFunctionType
F32, BF16, I32 = dt.float32, dt.bfloat16, dt.int32
BF = ml_dtypes.bfloat16

B, N, D = 2, 2048, 1024
H, DH = 16, 64
HPC = 4            # heads per core
FPC = HPC * DH     # 256 inner features per core
KC = D // 128      # 8 contraction chunks
FC = FPC // 128    # 2 feature chunks per core
NJ = N // 512      # 4 gather chunks (AllGather slice granularity)
NT = N // 128      # 16 m-tiles
NJ2 = N // 1024    # 2 compute chunks
EPS = 1e-6
CORES = 8
GROUPS4 = [[0, 1, 2, 3], [4, 5, 6, 7]]
GROUP8 = [list(range(CORES))]

_CACHED_NC = None


def build(dbg=False):
    nc = bacc.Bacc("TRN2", target_bir_lowering=False, debug=False, num_devices=CORES)

    xT = nc.dram_tensor("xT", [128, KC, N], BF16, kind="ExternalInput")
    wq_d = nc.dram_tensor("wq", [128, KC, FPC], BF16, kind="ExternalInput")
    wk_d = nc.dram_tensor("wk", [128, KC, FPC], BF16, kind="ExternalInput")
    wv_d = nc.dram_tensor("wv", [128, KC, FPC], BF16, kind="ExternalInput")
    wo_d = nc.dram_tensor("wo", [128, KC, D], BF16, kind="ExternalInput")
    bo_d = nc.dram_tensor("bo", [1, D], BF16, kind="ExternalInput")
    wqc_d = nc.dram_tensor("wqc", [128, FC], BF16, kind="ExternalInput")
    wkc_d = nc.dram_tensor("wkc", [128, FC], BF16, kind="ExternalInput")
    cos_d = nc.dram_tensor("cos_t", [128, N], BF16, kind="ExternalInput")
    sin_d = nc.dram_tensor("sin_t", [128, N], BF16, kind="ExternalInput")
    gidx_d = nc.dram_tensor("gidx", [128, KC], I32, kind="ExternalInput")
    out_d = nc.dram_tensor("out", [512, D], F32, kind="ExternalOutput")

    if dbg:
        dbg_qn = nc.dram_tensor("dbg_qn", [128, FC, N], BF16, kind="ExternalOutput")
        dbg_kt = nc.dram_tensor("dbg_kt", [128, FC, N], BF16, kind="ExternalOutput")
        dbg_ri = nc.dram_tensor("dbg_ri", [33, N], F32, kind="ExternalOutput")
        dbg_rm = nc.dram_tensor("dbg_rm", [128, NT], F32, kind="ExternalOutput")
        dbg_va = nc.dram_tensor(
            "dbg_va", [128, NT, HPC, DH + 1], BF16, kind="ExternalOutput"
        )
        dbg_osb = nc.dram_tensor("dbg_osb", [128, FC, N], BF16, kind="ExternalOutput")
        dbg_og = nc.dram_tensor("dbg_og", [128, KC, 512], BF16, kind="ExternalOutput")
        dbg_pt = nc.dram_tensor("dbg_pt", [128, 1024], BF16, kind="ExternalOutput")
        dbg_rec = nc.dram_tensor("dbg_rec", [HPC * 2, 1024], F32, kind="ExternalOutput")

    # collective bounce buffers (ssq AllReduce split into two n-halves so the
    # first fires mid phase A and both overlap compute)
    ssq_in = [nc.dram_tensor(f"ssq_in{i}", [2, N // 2], F32) for i in range(2)]
    ssq_out = [nc.dram_tensor(f"ssq_out{i}", [2, N // 2], F32) for i in range(2)]
    ag_in = [nc.dram_tensor(f"ag_in{c}", [128, N], BF16) for c in range(FC)]
    ag_out = [
        nc.dram_tensor(f"ag_out{c}", [CORES * 128, N], BF16, addr_space="Shared")
        for c in range(FC)
    ]

    with tile.TileContext(nc) as tc:
        with tc.tile_pool(name="persist", bufs=1) as pp:
            # ---- constants ------------------------------------------------
            ones_col32 = pp.tile([128, 1], F32, tag="onesc32")
            nc.gpsimd.memset(ones_col32[:], 1.0)
            ones_col_bf = pp.tile([128, 1], BF16, tag="onescbf")
            nc.vector.tensor_copy(ones_col_bf[:], ones_col32[:])
            ones_row32 = pp.tile([1, 128], F32, tag="onesr32")
            nc.gpsimd.memset(ones_row32[:], 1.0)
            ones_row_bf = pp.tile([1, 128], BF16, tag="onesrbf")
            nc.vector.tensor_copy(ones_row_bf[:], ones_row32[:])
            one1_f32 = pp.tile([33, 1], F32, tag="one1")
            nc.gpsimd.memset(one1_f32[:], 1.0)
            # activation bias values at consumer base partitions
            eps_t = pp.tile([33, 1], F32, tag="eps")
            nc.gpsimd.memset(eps_t[:], EPS)
            bexp_t = pp.tile([33, 1], F32, tag="bexp")
            nc.gpsimd.memset(bexp_t[:], 0.0)
            nc.gpsimd.memset(bexp_t[32:33, :], -float(np.log(8.0)))

            wqc_sb = pp.tile([128, FC], BF16, tag="wqc")
            wkc_sb = pp.tile([128, FC], BF16, tag="wkc")
            nc.sync.dma_start(out=wqc_sb[:], in_=wqc_d[:])
            nc.sync.dma_start(out=wkc_sb[:], in_=wkc_d[:])
            gidx_sb = pp.tile([128, KC], I32, tag="gidx")
            nc.sync.dma_start(out=gidx_sb[:], in_=gidx_d[:])

            # ---- big persistent tensors ----------------------------------
            # DMA order matters: the first q/k matmul group needs xsb j=0 and
            # wq only, so those go first on the queue
            xsb = pp.tile([128, KC, N], BF16, tag="xsb")
            wq_sb = pp.tile([128, KC, FPC], BF16, tag="wq")
            wk_sb = pp.tile([128, KC, FPC], BF16, tag="wk")
            wv_sb = pp.tile([128, KC, FPC], BF16, tag="wv")
            cos_sb = pp.tile([128, N], BF16, tag="cos")
            sin_sb = pp.tile([128, N], BF16, tag="sin")
            nc.sync.dma_start(out=xsb[:, :, 0:512], in_=xT[:, :, 0:512])
            nc.sync.dma_start(out=wq_sb[:], in_=wq_d[:])
            nc.sync.dma_start(out=cos_sb[:, 0:512], in_=cos_d[:, 0:512])
            nc.sync.dma_start(out=sin_sb[:, 0:512], in_=sin_d[:, 0:512])
            nc.sync.dma_start(out=wk_sb[:], in_=wk_d[:])
            for j in range(1, NJ):
                jsl = slice(j * 512, (j + 1) * 512)
                nc.sync.dma_start(out=xsb[:, :, jsl], in_=xT[:, :, jsl])
                nc.sync.dma_start(out=cos_sb[:, jsl], in_=cos_d[:, jsl])
                nc.sync.dma_start(out=sin_sb[:, jsl], in_=sin_d[:, jsl])
            nc.sync.dma_start(out=wv_sb[:], in_=wv_d[:])
            wo_sb = pp.tile([128, KC, D], BF16, tag="wo")
            nc.gpsimd.dma_start(out=wo_sb[:], in_=wo_d[:])
            bo_sb = pp.tile([1, D], BF16, tag="bo")
            nc.gpsimd.dma_start(out=bo_sb[:], in_=bo_d[:])

            kT = pp.tile([128, FC, N], BF16, tag="kT")
            qn = pp.tile([128, FC, N], BF16, tag="qn")
            v_aug = pp.tile([128, NT, HPC, DH + 1], BF16, tag="vaug")
            nc.vector.tensor_copy(
                v_aug[:, :, :, DH : DH + 1],
                ones_col32[:].to_broadcast([128, NT, HPC, 1]),
            )
            o_sb = pp.tile([128, FC, N], BF16, tag="osb")
            rmsk_col = pp.tile([128, NT], F32, tag="rmskcol")
            og = pp.tile([128, KC, 512], BF16, tag="og")

            # ================= PHASE A: q/k projections + rope =============
            with (
                tc.tile_pool(name="pa", bufs=1) as pa,
                tc.tile_pool(name="pwa", bufs=4) as pwa,
            ):
                # pair-swap permutation matrix: psw[p, 2f+e] = 1 iff p == 2f+1-e
                psw32 = pa.tile([128, 128], F32, tag="psw32")
                nc.gpsimd.memset(psw32[:], 0.0)
                nc.gpsimd.affine_select(
                    out=psw32[:].rearrange("p (f e) -> p f e", e=2),
                    in_=psw32[:].rearrange("p (f e) -> p f e", e=2),
                    compare_op=mybir.AluOpType.not_equal,
                    fill=1.0,
                    base=-1,
                    pattern=[[-2, 64], [1, 2]],
                    channel_multiplier=1,
                )
                psw = pa.tile([128, 128], BF16, tag="psw")
                nc.vector.tensor_copy(psw[:], psw32[:])

                qpre = pa.tile([128, FC, N], BF16, tag="qpre")
                # row-vector stripes at base partitions 0 (q) and 32 (k)
                ssqp = pa.tile([33, N], F32, tag="ssqp")
                sq2 = pa.tile([33, N], F32, tag="sq2")
                lnv = pa.tile([33, N], F32, tag="lnv")
                rinv = pp.tile([33, N], F32, tag="rinv")
                bq = pa.tile([128, N], F32, tag="bq")
                # rows 1-31 are never written by the ssq path but are read by
                # the combined [33, N] ln/exp below; keep them finite
                nc.gpsimd.memset(sq2[:], 1.0)

                mul = mybir.AluOpType.mult
                with (
                    tc.tile_pool(name="psA", bufs=3, space="PSUM") as psA,
                    tc.tile_pool(name="psSw", bufs=2, space="PSUM") as psSw,
                    tc.tile_pool(name="psS", bufs=2, space="PSUM") as psS,
                ):
                  for j in range(NJ):
                    jsl = slice(j * 512, (j + 1) * 512)
                    for ti, (w_sb, wcol, dest) in enumerate((
                        (wq_sb, wqc_sb, qpre),
                        (wk_sb, wkc_sb, kT),
                    )):
                        ssq_ps = psS.tile([1, 512], F32, tag="ssq", name=f"ssq{j}{ti}")
                        for fc in range(FC):
                            fsl = slice(fc * 128, (fc + 1) * 128)
                            prj = psA.tile(
                                [128, 512], F32, tag="proj", name=f"prj{j}{ti}{fc}"
                            )
                            for kc in range(KC):
                                nc.tensor.matmul(
                                    prj[:],
                                    w_sb[:, kc, fsl],
                                    xsb[:, kc, jsl],
                                    start=(kc == 0),
                                    stop=(kc == KC - 1),
                                )
                            qw = pwa.tile([128, 512], BF16, tag="qw", name=f"qw{j}{ti}{fc}")
                            nc.scalar.activation(qw[:], prj[:], AF.Copy)
                            # sum-of-squares partial (DVE: only one PSUM input)
                            q2 = pwa.tile([128, 512], BF16, tag="q2", name=f"q2_{j}{ti}{fc}")
                            nc.vector.tensor_mul(q2[:], prj[:], qw[:])
                            nc.tensor.matmul(
                                ssq_ps[:],
                                ones_col_bf[:],
                                q2[:],
                                start=(fc == 0),
                                stop=(fc == FC - 1),
                            )
                            # rope with norm weight folded in
                            tcos = pwa.tile([128, 512], BF16, tag="tcos", name=f"tc{j}{ti}{fc}")
                            nc.vector.scalar_tensor_tensor(
                                tcos[:], qw[:], wcol[:, fc : fc + 1], cos_sb[:, jsl],
                                op0=mul, op1=mul,
                            )
                            tsin = pwa.tile([128, 512], BF16, tag="tsin", name=f"ts{j}{ti}{fc}")
                            nc.vector.scalar_tensor_tensor(
                                tsin[:], qw[:], wcol[:, fc : fc + 1], sin_sb[:, jsl],
                                op0=mul, op1=mul,
                            )
                            swp = psSw.tile([128, 512], F32, tag="swp", name=f"sw{j}{ti}{fc}")
                            nc.tensor.matmul(swp[:], psw[:], tsin[:], start=True, stop=True)
                            nc.vector.tensor_add(dest[:, fc, jsl], tcos[:], swp[:])
                        nc.scalar.activation(
                            ssqp[32 * ti : 32 * ti + 1, jsl], ssq_ps[:], AF.Copy
                        )
                    if j % 2 == 1:
                        # fire this n-half's ssq AllReduce; its latency hides
                        # behind the remaining q/k (half 0) or v (half 1) work
                        i = j // 2
                        hsl = slice(i * 1024, (i + 1) * 1024)
                        nc.sync.dma_start(out=ssq_in[i][0:1, :], in_=ssqp[0:1, hsl])
                        nc.sync.dma_start(out=ssq_in[i][1:2, :], in_=ssqp[32:33, hsl])
                        nc.gpsimd.collective_compute(
                            "AllReduce",
                            mybir.AluOpType.add,
                            replica_groups=GROUPS4,
                            ins=[ssq_in[i][:]],
                            outs=[ssq_out[i][:]],
                        )
                        nc.sync.dma_start(out=sq2[0:1, hsl], in_=ssq_out[i][0:1, :])
                        nc.sync.dma_start(out=sq2[32:33, hsl], in_=ssq_out[i][1:2, :])

                # rq = rsqrt(ssq/D + eps) = exp(-0.5*ln(ssq/D + eps))
                # rk8 = rq_k / 8      (score scale folded in, bias = -ln 8)
                # (both AllReduce halves were already triggered inside the j
                # loop; half 0 completed during j=2/3 so ln/exp-a is instant)
                for i in range(2):
                    hsl = slice(i * 1024, (i + 1) * 1024)
                    nc.scalar.activation(
                        lnv[:, hsl], sq2[:, hsl], AF.Ln, scale=1.0 / D,
                        bias=eps_t[:],
                    )
                    nc.scalar.activation(
                        rinv[:, hsl], lnv[:, hsl], AF.Exp, scale=-0.5,
                        bias=bexp_t[:],
                    )

                # ---- v projections (overlap the AllReduce tail) -----------
                with tc.tile_pool(name="psV", bufs=4, space="PSUM") as psV:
                    for nt in range(NT):
                        nsl = slice((nt % 8) * 128, (nt % 8 + 1) * 128)
                        jsl = slice((nt // 8) * 1024, (nt // 8) * 1024 + 1024)
                        vps = psV.tile([128, FPC], F32, tag="v", name=f"v{nt}")
                        for kc in range(KC):
                            nc.tensor.matmul(
                                vps[:],
                                xsb[:, kc, jsl][:, nsl],
                                wv_sb[:, kc, :],
                                start=(kc == 0),
                                stop=(kc == KC - 1),
                            )
                        nc.scalar.activation(
                            v_aug[:, nt, :, 0:DH],
                            vps[:].rearrange("p (h f) -> p h f", f=DH),
                            AF.Copy,
                        )

                    # k-scales into column form [128, 16] via 16 rank-1 matmuls
                    rk_ps = psV.tile([128, NT], F32, tag="v", name="rkps")
                    for t in range(NT):
                        nc.tensor.matmul(
                            rk_ps[:, t : t + 1],
                            rinv[32:33, t * 128 : (t + 1) * 128],
                            one1_f32[32:33, :],
                            start=True,
                            stop=True,
                        )
                    nc.vector.tensor_copy(rmsk_col[:], rk_ps[:])

                # qn = qpre * broadcast(rq)
                for i in range(2):
                    hsl = slice(i * 1024, (i + 1) * 1024)
                    nc.gpsimd.partition_broadcast(bq[:, hsl], rinv[0:1, hsl])
                    for fc in range(FC):
                        nc.vector.tensor_mul(
                            qn[:, fc, hsl], qpre[:, fc, hsl], bq[:, hsl]
                        )

            # ============== PHASE B: attention, per head ===================
            with (
                tc.tile_pool(name="pwb", bufs=4) as pwb,
                tc.tile_pool(name="ppb", bufs=3) as ppb,
                tc.tile_pool(name="psB", bufs=2, space="PSUM") as psB,
                tc.tile_pool(name="psOV", bufs=2, space="PSUM") as psOV,
            ):
                for h in range(HPC):
                    ch = h // 2
                    po = 64 * (h % 2)
                    ov = [
                        psOV.tile([DH + 1, 1024], F32, tag="ov", name=f"ov{h}_{i}")
                        for i in range(NJ2)
                    ]
                    units = [(mt, hf) for mt in range(NT) for hf in range(NJ2)]

                    def emit_scores(k):
                        mt, hf = units[k]
                        s_ps = psB.tile(
                            [128, 1024], F32, tag="S", name=f"S{h}_{mt}_{hf}"
                        )
                        # col-packed pairs: m-halves go to array col groups
                        # 0/64 concurrently (tile_position auto-derived)
                        for j2 in range(2):
                            jsl = slice(
                                hf * 1024 + j2 * 512, hf * 1024 + j2 * 512 + 512
                            )
                            for mh in range(2):
                                msl = slice(
                                    mt * 128 + mh * 64, mt * 128 + mh * 64 + 64
                                )
                                nc.tensor.matmul(
                                    s_ps[mh * 64 : (mh + 1) * 64,
                                         j2 * 512 : (j2 + 1) * 512],
                                    kT[po : po + 64, ch, msl],
                                    qn[po : po + 64, ch, jsl],
                                    start=True,
                                    stop=True,
                                )
                        return s_ps

                    def emit_expov(k, s_ps):
                        mt, hf = units[k]
                        p_t = ppb.tile(
                            [128, 1024], BF16, tag="P", name=f"P{h}_{mt}_{hf}"
                        )
                        nc.scalar.activation(
                            p_t[:], s_ps[:], AF.Exp,
                            scale=rmsk_col[:, mt : mt + 1],
                        )
                        if dbg and h == 0 and mt == 0 and hf == 0:
                            nc.sync.dma_start(out=dbg_pt[:], in_=p_t[:])
                        for j2 in range(2):
                            nc.tensor.matmul(
                                ov[hf][:, j2 * 512 : (j2 + 1) * 512],
                                v_aug[:, mt, h, :],
                                p_t[:, j2 * 512 : (j2 + 1) * 512],
                                start=(mt == 0),
                                stop=(mt == NT - 1),
                            )

                    # software pipeline: scores(k+1) is emitted before ov(k)
                    # so the in-order PE queue never stalls on the exp
                    prev = None
                    for k in range(len(units)):
                        cur = emit_scores(k)
                        if prev is not None:
                            emit_expov(k - 1, prev)
                        prev = cur
                    emit_expov(len(units) - 1, prev)

                    for hf in range(NJ2):
                        jsl = slice(hf * 1024, (hf + 1) * 1024)
                        # denominator row: psum partition 64 -> sbuf partition 0
                        # (DVE-only tail keeps the ACT queue free for exps)
                        den0 = pwb.tile([1, 1024], F32, tag="den0", name=f"d0{h}{hf}")
                        nc.vector.tensor_copy(den0[:], ov[hf][DH : DH + 1, :])
                        rec0 = pwb.tile([1, 1024], F32, tag="rec0", name=f"r0{h}{hf}")
                        nc.vector.reciprocal_approx_fast(rec0[:], den0[:])
                        if dbg:
                            nc.sync.dma_start(
                                out=dbg_rec[2 * h + hf : 2 * h + hf + 1, :], in_=rec0[:]
                            )
                        bcast = pwb.tile([DH, 1024], F32, tag="bcast", name=f"bc{h}{hf}")
                        nc.gpsimd.partition_broadcast(bcast[:], rec0[:])
                        nc.vector.tensor_mul(
                            o_sb[po : po + 64, ch, jsl], ov[hf][0:DH, :], bcast[:]
                        )
                    if h == 3:
                        # ch0 gather slices (AG0 completed during h2/h3, so
                        # these run instantly and never block the tail chain);
                        # phase C even-kc matmuls then overlap AG1's flight
                        for kc in range(0, KC, FC):
                            nc.gpsimd.indirect_dma_start(
                                out=og[:, kc, :],
                                out_offset=None,
                                in_=ag_out[0][:].rearrange("r (j n) -> (r j) n", n=512),
                                in_offset=bass.IndirectOffsetOnAxis(
                                    ap=gidx_sb[:, kc : kc + 1], axis=0
                                ),
                            )
                    if h % 2 == 1:
                        nc.sync.dma_start(out=ag_in[ch][:], in_=o_sb[:, ch, :])
                        nc.gpsimd.collective_compute(
                            "AllGather",
                            mybir.AluOpType.bypass,
                            replica_groups=GROUP8,
                            ins=[ag_in[ch][:]],
                            outs=[ag_out[ch][:]],
                        )

                # ch1 gathers wait on AG1 (the gpsimd queue is done with all
                # per-head work by now, so the wait blocks nothing)
                for kc in range(1, KC, FC):
                    nc.gpsimd.indirect_dma_start(
                        out=og[:, kc, :],
                        out_offset=None,
                        in_=ag_out[1][:].rearrange("r (j n) -> (r j) n", n=512),
                        in_offset=bass.IndirectOffsetOnAxis(
                            ap=gidx_sb[:, kc : kc + 1], axis=0
                        ),
                    )

            if dbg:
                nc.gpsimd.dma_start(out=dbg_qn[:], in_=qn[:])
                nc.gpsimd.dma_start(out=dbg_kt[:], in_=kT[:])
                nc.sync.dma_start(out=dbg_ri[:], in_=rinv[:])
                nc.sync.dma_start(out=dbg_rm[:], in_=rmsk_col[:])
                nc.sync.dma_start(out=dbg_va[:], in_=v_aug[:])
                nc.sync.dma_start(out=dbg_osb[:], in_=o_sb[:])
                nc.sync.dma_start(out=dbg_og[:], in_=og[:])

            # ========= PHASE C: output projection ==========================
            # even (ch 0) contractions first: they are ready while the ch 1
            # AllGather is still in flight
            kc_order = list(range(0, KC, 2)) + list(range(1, KC, 2))
            with (
                tc.tile_pool(name="pc", bufs=2) as pc,
                tc.tile_pool(name="psC", bufs=2, space="PSUM") as psC,
            ):
                for ntl in range(4):
                    yps = psC.tile([128, D], F32, tag="y", name=f"y{ntl}")
                    for dc in range(2):
                        dsl = slice(dc * 512, (dc + 1) * 512)
                        for i, kc in enumerate(kc_order):
                            nc.tensor.matmul(
                                yps[:, dsl],
                                og[:, kc, ntl * 128 : (ntl + 1) * 128],
                                wo_sb[:, kc, dsl],
                                start=(i == 0),
                                stop=False,
                            )
                        nc.tensor.matmul(
                            yps[:, dsl],
                            ones_row_bf[:],
                            bo_sb[:, dsl],
                            start=False,
                            stop=True,
                        )
                    ysb = pc.tile([128, D], F32, tag="ysb", name=f"ysb{ntl}")
                    nc.vector.tensor_copy(ysb[:], yps[:])
                    nc.sync.dma_start(
                        out=out_d[ntl * 128 : (ntl + 1) * 128, :], in_=ysb[:]
                    )

    nc.compile()
    return nc


def _rope_tables():
    """cos/sin tables matching the reference's f32 angle computation.

    C[d, n] = cos(n * theta[d//2]);  Ssw[2i] = +sin, Ssw[2i+1] = -sin
    (Ssw is the swapped-operand multiplier: rope = x*C + swap(x*Ssw)).
    Tiled x2 along partitions to cover a 2-head (128-row) chunk.
    """
    i2 = np.arange(0, DH, 2, dtype=np.float32)
    theta = (1.0 / (10000.0 ** (i2 / DH))).astype(np.float32)  # (32,)
    ang = np.arange(N, dtype=np.float32)[:, None] * theta[None, :]  # (N, 32) f32
    cos = np.cos(ang.astype(np.float64)).astype(np.float32).T  # (32, N)
    sin = np.sin(ang.astype(np.float64)).astype(np.float32).T
    cos_d = np.repeat(cos, 2, axis=0)  # (64, N)
    ssw = np.repeat(sin, 2, axis=0)
    ssw[1::2, :] *= -1.0
    cos_t = np.tile(cos_d, (2, 1)).astype(np.float32)  # (128, N)
    sin_t = np.tile(ssw, (2, 1)).astype(np.float32)
    return cos_t, sin_t


def _rearr(w):
    # [D, F] -> [128, KC, F] grouping the contraction dim into 128-row chunks
    d, f = w.shape
    return np.ascontiguousarray(
        w.reshape(KC, 128, f).transpose(1, 0, 2).astype(BF)
    )


def kernel(x, Wq, Wkv, norm_q_w, norm_k_w, Wo, bo, _trace=False, _dbg=False):
    global _CACHED_NC
    x = np.asarray(x, dtype=np.float32)
    Wq = np.asarray(Wq, dtype=np.float32)
    Wkv = np.asarray(Wkv, dtype=np.float32)
    norm_q_w = np.asarray(norm_q_w, dtype=np.float32)
    norm_k_w = np.asarray(norm_k_w, dtype=np.float32)
    Wo = np.asarray(Wo, dtype=np.float32)
    bo = np.asarray(bo, dtype=np.float32)

    cos_t, sin_t = _rope_tables()
    if _dbg:
        nc = build(dbg=True)
    else:
        if _CACHED_NC is None:
            _CACHED_NC = build()
        nc = _CACHED_NC

    in_maps = []
    for c in range(CORES):
        b, g = c // 4, c % 4
        fsl = slice(g * FPC, (g + 1) * FPC)
        gidx = np.empty((128, KC), dtype=np.int32)
        p = np.arange(128)
        for kc in range(KC):
            gidx[:, kc] = (128 * (4 * b + kc // FC) + p) * NJ + g
        in_maps.append(
            {
                "xT": _rearr(np.ascontiguousarray(x[b].T)),
                "wq": _rearr(Wq[:, fsl]),
                "wk": _rearr(Wkv[:, fsl]),
                "wv": _rearr(Wkv[:, D + g * FPC : D + (g + 1) * FPC]),
                "wo": _rearr(Wo),
                "bo": bo.reshape(1, D).astype(BF),
                "wqc": np.ascontiguousarray(
                    norm_q_w[fsl].reshape(FC, 128).T
                ).astype(BF),
                "wkc": np.ascontiguousarray(
                    norm_k_w[fsl].reshape(FC, 128).T
                ).astype(BF),
                "cos_t": cos_t.astype(BF),
                "sin_t": sin_t.astype(BF),
                "gidx": gidx,
            }
        )

    res = run_bass_kernel_spmd(nc, in_maps, list(range(CORES)), trace=_trace)
    out = np.empty((B, N, D), dtype=np.float32)
    for c in range(CORES):
        b, g = c // 4, c % 4
        out[b, g * 512 : (g + 1) * 512, :] = res.results[c]["out"]
    if _trace or _dbg:
        return out, res
    return out


# revision 40
# speedup vs baseline: 1.6548x; 1.0413x over previous
"""Distributed Bass kernel for nn_Attention (B=2, N=2048, D=1024, H=16, DH=64) on 8 trn2 cores.

Sharding: data-parallel over batch (cores 0-3 -> b=0, 4-7 -> b=1), tensor-parallel
over heads (4 heads / 256 inner features per core).  v2 design (bf16 everywhere):
  all matmuls bf16 (weights/x pre-cast+rearranged on host, fp32 PSUM accumulate),
  q/k projections + rope first, ssq AllReduce overlapped with v projections,
  RMSNorm scales via ACT ln/exp (single activation table set for whole kernel),
  attention scores col-packed 2x via tile_position quadrants (DH=64),
  softmax denominator via ones-row in v, fast-approx reciprocal,
  8-core AllGather per 2-head chunk (bf16), out-projection ordered so the
  second AllGather overlaps the first half of the output matmuls.
Host assembles the (2, 2048, 1024) output from the 8 (512, 1024) shards.
"""
import os
import sys

for _p in ("/opt/trn_rl_repo", "/root/.axon_site/_ro/trn_rl_repo"):
    if os.path.isdir(_p) and _p not in sys.path:
        sys.path.insert(0, _p)

import numpy as np
import ml_dtypes
import concourse.bass as bass
import concourse.mybir as mybir
import concourse.tile as tile
from concourse import bacc
from concourse.bass_utils import run_bass_kernel_spmd

dt = mybir.dt
AF = mybir.ActivationFunctionType
F32, BF16, I32 = dt.float32, dt.bfloat16, dt.int32
BF = ml_dtypes.bfloat16

B, N, D = 2, 2048, 1024
H, DH = 16, 64
HPC = 4            # heads per core
FPC = HPC * DH     # 256 inner features per core
KC = D // 128      # 8 contraction chunks
FC = FPC // 128    # 2 feature chunks per core
NJ = N // 512      # 4 gather chunks (AllGather slice granularity)
NT = N // 128      # 16 m-tiles
NJ2 = N // 1024    # 2 compute chunks
EPS = 1e-6
CORES = 8
GROUPS4 = [[0, 1, 2, 3], [4, 5, 6, 7]]
GROUP8 = [list(range(CORES))]

_CACHED_NC = None


def build(dbg=False):
    nc = bacc.Bacc("TRN2", target_bir_lowering=False, debug=False, num_devices=CORES)

    xT = nc.dram_tensor("xT", [128, KC, N], BF16, kind="ExternalInput")
    wq_d = nc.dram_tensor("wq", [128, KC, FPC], BF16, kind="ExternalInput")
    wk_d = nc.dram_tensor("wk", [128, KC, FPC], BF16, kind="ExternalInput")
    wv_d = nc.dram_tensor("wv", [128, KC, FPC], BF16, kind="ExternalInput")
    wo_d = nc.dram_tensor("wo", [128, KC, D], BF16, kind="ExternalInput")
    bo_d = nc.dram_tensor("bo", [1, D], BF16, kind="ExternalInput")
    wqc_d = nc.dram_tensor("wqc", [128, FC], BF16, kind="ExternalInput")
    wkc_d = nc.dram_tensor("wkc", [128, FC], BF16, kind="ExternalInput")
    cos_d = nc.dram_tensor("cos_t", [128, N], BF16, kind="ExternalInput")
    sin_d = nc.dram_tensor("sin_t", [128, N], BF16, kind="ExternalInput")
    gidx_d = nc.dram_tensor("gidx", [128, KC], I32, kind="ExternalInput")
    out_d = nc.dram_tensor("out", [512, D], F32, kind="ExternalOutput")

    if dbg:
        dbg_qn = nc.dram_tensor("dbg_qn", [128, FC, N], BF16, kind="ExternalOutput")
        dbg_kt = nc.dram_tensor("dbg_kt", [128, FC, N], BF16, kind="ExternalOutput")
        dbg_ri = nc.dram_tensor("dbg_ri", [33, N], F32, kind="ExternalOutput")
        dbg_rm = nc.dram_tensor("dbg_rm", [128, NT], F32, kind="ExternalOutput")
        dbg_va = nc.dram_tensor(
            "dbg_va", [128, NT, HPC, DH + 1], BF16, kind="ExternalOutput"
        )
        dbg_osb = nc.dram_tensor("dbg_osb", [128, FC, N], BF16, kind="ExternalOutput")
        dbg_og = nc.dram_tensor("dbg_og", [128, KC, 512], BF16, kind="ExternalOutput")
        dbg_pt = nc.dram_tensor("dbg_pt", [128, 1024], BF16, kind="ExternalOutput")
        dbg_rec = nc.dram_tensor("dbg_rec", [HPC * 2, 1024], F32, kind="ExternalOutput")

    # collective bounce buffers (ssq AllReduce split into two n-halves so the
    # first fires mid phase A and both overlap compute)
    ssq_in = [nc.dram_tensor(f"ssq_in{i}", [2, N // 2], F32) for i in range(2)]
    ssq_out = [nc.dram_tensor(f"ssq_out{i}", [2, N // 2], F32) for i in range(2)]
    ag_in = [nc.dram_tensor(f"ag_in{c}", [128, N], BF16) for c in range(FC)]
    ag_out = [
        nc.dram_tensor(f"ag_out{c}", [CORES * 128, N], BF16, addr_space="Shared")
        for c in range(FC)
    ]

    with tile.TileContext(nc) as tc:
        with tc.tile_pool(name="persist", bufs=1) as pp:
            # ---- constants ------------------------------------------------
            ones_col32 = pp.tile([128, 1], F32, tag="onesc32")
            nc.gpsimd.memset(ones_col32[:], 1.0)
            ones_col_bf = pp.tile([128, 1], BF16, tag="onescbf")
            nc.vector.tensor_copy(ones_col_bf[:], ones_col32[:])
            ones_row32 = pp.tile([1, 128], F32, tag="onesr32")
            nc.gpsimd.memset(ones_row32[:], 1.0)
            ones_row_bf = pp.tile([1, 128], BF16, tag="onesrbf")
            nc.vector.tensor_copy(ones_row_bf[:], ones_row32[:])
            one1_f32 = pp.tile([33, 1], F32, tag="one1")
            nc.gpsimd.memset(one1_f32[:], 1.0)
            # activation bias values at consumer base partitions
            eps_t = pp.tile([33, 1], F32, tag="eps")
            nc.gpsimd.memset(eps_t[:], EPS)
            bexp_t = pp.tile([33, 1], F32, tag="bexp")
            nc.gpsimd.memset(bexp_t[:], 0.0)
            nc.gpsimd.memset(bexp_t[32:33, :], -float(np.log(8.0)))

            wqc_sb = pp.tile([128, FC], BF16, tag="wqc")
            wkc_sb = pp.tile([128, FC], BF16, tag="wkc")
            nc.sync.dma_start(out=wqc_sb[:], in_=wqc_d[:])
            nc.sync.dma_start(out=wkc_sb[:], in_=wkc_d[:])
            gidx_sb = pp.tile([128, KC], I32, tag="gidx")
            nc.sync.dma_start(out=gidx_sb[:], in_=gidx_d[:])

            # ---- big persistent tensors ----------------------------------
            # DMA order matters: the first q/k matmul group needs xsb j=0 and
            # wq only, so those go first on the queue
            xsb = pp.tile([128, KC, N], BF16, tag="xsb")
            wq_sb = pp.tile([128, KC, FPC], BF16, tag="wq")
            wk_sb = pp.tile([128, KC, FPC], BF16, tag="wk")
            wv_sb = pp.tile([128, KC, FPC], BF16, tag="wv")
            cos_sb = pp.tile([128, N], BF16, tag="cos")
            sin_sb = pp.tile([128, N], BF16, tag="sin")
            nc.sync.dma_start(out=xsb[:, :, 0:512], in_=xT[:, :, 0:512])
            nc.scalar.dma_start(out=wq_sb[:], in_=wq_d[:])
            nc.scalar.dma_start(out=cos_sb[:, 0:512], in_=cos_d[:, 0:512])
            nc.scalar.dma_start(out=sin_sb[:, 0:512], in_=sin_d[:, 0:512])
            nc.scalar.dma_start(out=wk_sb[:], in_=wk_d[:])
            for j in range(1, NJ):
                jsl = slice(j * 512, (j + 1) * 512)
                nc.sync.dma_start(out=xsb[:, :, jsl], in_=xT[:, :, jsl])
                nc.scalar.dma_start(out=cos_sb[:, jsl], in_=cos_d[:, jsl])
                nc.scalar.dma_start(out=sin_sb[:, jsl], in_=sin_d[:, jsl])
            nc.scalar.dma_start(out=wv_sb[:], in_=wv_d[:])
            wo_sb = pp.tile([128, KC, D], BF16, tag="wo")
            nc.gpsimd.dma_start(out=wo_sb[:], in_=wo_d[:])
            bo_sb = pp.tile([1, D], BF16, tag="bo")
            nc.gpsimd.dma_start(out=bo_sb[:], in_=bo_d[:])

            kT = pp.tile([128, FC, N], BF16, tag="kT")
            qn = pp.tile([128, FC, N], BF16, tag="qn")
            v_aug = pp.tile([128, NT, HPC, DH + 1], BF16, tag="vaug")
            nc.vector.tensor_copy(
                v_aug[:, :, :, DH : DH + 1],
                ones_col32[:].to_broadcast([128, NT, HPC, 1]),
            )
            o_sb = pp.tile([128, FC, N], BF16, tag="osb")
            rmsk_col = pp.tile([128, NT], F32, tag="rmskcol")
            og = pp.tile([128, KC, 512], BF16, tag="og")

            # ================= PHASE A: q/k projections + rope =============
            with (
                tc.tile_pool(name="pa", bufs=1) as pa,
                tc.tile_pool(name="pwa", bufs=4) as pwa,
            ):
                # pair-swap permutation matrix: psw[p, 2f+e] = 1 iff p == 2f+1-e
                psw32 = pa.tile([128, 128], F32, tag="psw32")
                nc.gpsimd.memset(psw32[:], 0.0)
                nc.gpsimd.affine_select(
                    out=psw32[:].rearrange("p (f e) -> p f e", e=2),
                    in_=psw32[:].rearrange("p (f e) -> p f e", e=2),
                    compare_op=mybir.AluOpType.not_equal,
                    fill=1.0,
                    base=-1,
                    pattern=[[-2, 64], [1, 2]],
                    channel_multiplier=1,
                )
                psw = pa.tile([128, 128], BF16, tag="psw")
                nc.vector.tensor_copy(psw[:], psw32[:])

                qpre = pa.tile([128, FC, N], BF16, tag="qpre")
                # row-vector stripes at base partitions 0 (q) and 32 (k)
                ssqp = pa.tile([33, N], F32, tag="ssqp")
                sq2 = pa.tile([33, N], F32, tag="sq2")
                lnv = pa.tile([33, N], F32, tag="lnv")
                rinv = pp.tile([33, N], F32, tag="rinv")
                bq = pa.tile([128, N], F32, tag="bq")
                # rows 1-31 are never written by the ssq path but are read by
                # the combined [33, N] ln/exp below; keep them finite
                nc.gpsimd.memset(sq2[:], 1.0)

                mul = mybir.AluOpType.mult
                with (
                    tc.tile_pool(name="psA", bufs=3, space="PSUM") as psA,
                    tc.tile_pool(name="psSw", bufs=2, space="PSUM") as psSw,
                    tc.tile_pool(name="psS", bufs=2, space="PSUM") as psS,
                ):
                  for j in range(NJ):
                    jsl = slice(j * 512, (j + 1) * 512)
                    for ti, (w_sb, wcol, dest) in enumerate((
                        (wq_sb, wqc_sb, qpre),
                        (wk_sb, wkc_sb, kT),
                    )):
                        ssq_ps = psS.tile([1, 512], F32, tag="ssq", name=f"ssq{j}{ti}")
                        for fc in range(FC):
                            fsl = slice(fc * 128, (fc + 1) * 128)
                            prj = psA.tile(
                                [128, 512], F32, tag="proj", name=f"prj{j}{ti}{fc}"
                            )
                            for kc in range(KC):
                                nc.tensor.matmul(
                                    prj[:],
                                    w_sb[:, kc, fsl],
                                    xsb[:, kc, jsl],
                                    start=(kc == 0),
                                    stop=(kc == KC - 1),
                                )
                            qw = pwa.tile([128, 512], BF16, tag="qw", name=f"qw{j}{ti}{fc}")
                            nc.scalar.activation(qw[:], prj[:], AF.Copy)
                            # sum-of-squares partial (DVE: only one PSUM input)
                            q2 = pwa.tile([128, 512], BF16, tag="q2", name=f"q2_{j}{ti}{fc}")
                            nc.vector.tensor_mul(q2[:], prj[:], qw[:])
                            nc.tensor.matmul(
                                ssq_ps[:],
                                ones_col_bf[:],
                                q2[:],
                                start=(fc == 0),
                                stop=(fc == FC - 1),
                            )
                            # rope with norm weight folded in
                            tcos = pwa.tile([128, 512], BF16, tag="tcos", name=f"tc{j}{ti}{fc}")
                            nc.vector.scalar_tensor_tensor(
                                tcos[:], qw[:], wcol[:, fc : fc + 1], cos_sb[:, jsl],
                                op0=mul, op1=mul,
                            )
                            tsin = pwa.tile([128, 512], BF16, tag="tsin", name=f"ts{j}{ti}{fc}")
                            nc.vector.scalar_tensor_tensor(
                                tsin[:], qw[:], wcol[:, fc : fc + 1], sin_sb[:, jsl],
                                op0=mul, op1=mul,
                            )
                            swp = psSw.tile([128, 512], F32, tag="swp", name=f"sw{j}{ti}{fc}")
                            nc.tensor.matmul(swp[:], psw[:], tsin[:], start=True, stop=True)
                            nc.vector.tensor_add(dest[:, fc, jsl], tcos[:], swp[:])
                        nc.scalar.activation(
                            ssqp[32 * ti : 32 * ti + 1, jsl], ssq_ps[:], AF.Copy
                        )
                    if j % 2 == 1:
                        # fire this n-half's ssq AllReduce; its latency hides
                        # behind the remaining q/k (half 0) or v (half 1) work
                        i = j // 2
                        hsl = slice(i * 1024, (i + 1) * 1024)
                        nc.sync.dma_start(out=ssq_in[i][0:1, :], in_=ssqp[0:1, hsl])
                        nc.sync.dma_start(out=ssq_in[i][1:2, :], in_=ssqp[32:33, hsl])
                        nc.gpsimd.collective_compute(
                            "AllReduce",
                            mybir.AluOpType.add,
                            replica_groups=GROUPS4,
                            ins=[ssq_in[i][:]],
                            outs=[ssq_out[i][:]],
                        )
                        nc.sync.dma_start(out=sq2[0:1, hsl], in_=ssq_out[i][0:1, :])
                        nc.sync.dma_start(out=sq2[32:33, hsl], in_=ssq_out[i][1:2, :])

                # rq = rsqrt(ssq/D + eps) = exp(-0.5*ln(ssq/D + eps))
                # rk8 = rq_k / 8      (score scale folded in, bias = -ln 8)
                # (both AllReduce halves were already triggered inside the j
                # loop; half 0 completed during j=2/3 so ln/exp-a is instant)
                for i in range(2):
                    hsl = slice(i * 1024, (i + 1) * 1024)
                    nc.scalar.activation(
                        lnv[:, hsl], sq2[:, hsl], AF.Ln, scale=1.0 / D,
                        bias=eps_t[:],
                    )
                    nc.scalar.activation(
                        rinv[:, hsl], lnv[:, hsl], AF.Exp, scale=-0.5,
                        bias=bexp_t[:],
                    )

                # ---- v projections (overlap the AllReduce tail) -----------
                with tc.tile_pool(name="psV", bufs=4, space="PSUM") as psV:
                    for nt in range(NT):
                        nsl = slice((nt % 8) * 128, (nt % 8 + 1) * 128)
                        jsl = slice((nt // 8) * 1024, (nt // 8) * 1024 + 1024)
                        vps = psV.tile([128, FPC], F32, tag="v", name=f"v{nt}")
                        for kc in range(KC):
                            nc.tensor.matmul(
                                vps[:],
                                xsb[:, kc, jsl][:, nsl],
                                wv_sb[:, kc, :],
                                start=(kc == 0),
                                stop=(kc == KC - 1),
                            )
                        nc.scalar.activation(
                            v_aug[:, nt, :, 0:DH],
                            vps[:].rearrange("p (h f) -> p h f", f=DH),
                            AF.Copy,
                        )

                    # k-scales into column form [128, 16] via 16 rank-1 matmuls
                    rk_ps = psV.tile([128, NT], F32, tag="v", name="rkps")
                    for t in range(NT):
                        nc.tensor.matmul(
                            rk_ps[:, t : t + 1],
                            rinv[32:33, t * 128 : (t + 1) * 128],
                            one1_f32[32:33, :],
                            start=True,
                            stop=True,
                        )
                    nc.vector.tensor_copy(rmsk_col[:], rk_ps[:])

                # qn = qpre * broadcast(rq)
                for i in range(2):
                    hsl = slice(i * 1024, (i + 1) * 1024)
                    nc.gpsimd.partition_broadcast(bq[:, hsl], rinv[0:1, hsl])
                    for fc in range(FC):
                        nc.vector.tensor_mul(
                            qn[:, fc, hsl], qpre[:, fc, hsl], bq[:, hsl]
                        )

            # ============== PHASE B: attention, per head ===================
            with (
                tc.tile_pool(name="pwb", bufs=4) as pwb,
                tc.tile_pool(name="ppb", bufs=3) as ppb,
                tc.tile_pool(name="psB", bufs=2, space="PSUM") as psB,
                tc.tile_pool(name="psOV", bufs=2, space="PSUM") as psOV,
            ):
                for h in range(HPC):
                    ch = h // 2
                    po = 64 * (h % 2)
                    ov = [
                        psOV.tile([DH + 1, 1024], F32, tag="ov", name=f"ov{h}_{i}")
                        for i in range(NJ2)
                    ]
                    units = [(mt, hf) for mt in range(NT) for hf in range(NJ2)]

                    def emit_scores(k):
                        mt, hf = units[k]
                        s_ps = psB.tile(
                            [128, 1024], F32, tag="S", name=f"S{h}_{mt}_{hf}"
                        )
                        # col-packed pairs: m-halves go to array col groups
                        # 0/64 concurrently (tile_position auto-derived)
                        for j2 in range(2):
                            jsl = slice(
                                hf * 1024 + j2 * 512, hf * 1024 + j2 * 512 + 512
                            )
                            for mh in range(2):
                                msl = slice(
                                    mt * 128 + mh * 64, mt * 128 + mh * 64 + 64
                                )
                                nc.tensor.matmul(
                                    s_ps[mh * 64 : (mh + 1) * 64,
                                         j2 * 512 : (j2 + 1) * 512],
                                    kT[po : po + 64, ch, msl],
                                    qn[po : po + 64, ch, jsl],
                                    start=True,
                                    stop=True,
                                )
                        return s_ps

                    def emit_expov(k, s_ps):
                        mt, hf = units[k]
                        p_t = ppb.tile(
                            [128, 1024], BF16, tag="P", name=f"P{h}_{mt}_{hf}"
                        )
                        nc.scalar.activation(
                            p_t[:], s_ps[:], AF.Exp,
                            scale=rmsk_col[:, mt : mt + 1],
                        )
                        if dbg and h == 0 and mt == 0 and hf == 0:
                            nc.sync.dma_start(out=dbg_pt[:], in_=p_t[:])
                        for j2 in range(2):
                            nc.tensor.matmul(
                                ov[hf][:, j2 * 512 : (j2 + 1) * 512],
                                v_aug[:, mt, h, :],
                                p_t[:, j2 * 512 : (j2 + 1) * 512],
                                start=(mt == 0),
                                stop=(mt == NT - 1),
                            )

                    # software pipeline: scores(k+1) is emitted before ov(k)
                    # so the in-order PE queue never stalls on the exp
                    prev = None
                    for k in range(len(units)):
                        cur = emit_scores(k)
                        if prev is not None:
                            emit_expov(k - 1, prev)
                        prev = cur
                    emit_expov(len(units) - 1, prev)

                    for hf in range(NJ2):
                        jsl = slice(hf * 1024, (hf + 1) * 1024)
                        # denominator row: psum partition 64 -> sbuf partition 0
                        # (DVE-only tail keeps the ACT queue free for exps)
                        den0 = pwb.tile([1, 1024], F32, tag="den0", name=f"d0{h}{hf}")
                        nc.vector.tensor_copy(den0[:], ov[hf][DH : DH + 1, :])
                        rec0 = pwb.tile([1, 1024], F32, tag="rec0", name=f"r0{h}{hf}")
                        nc.vector.reciprocal_approx_fast(rec0[:], den0[:])
                        if dbg:
                            nc.sync.dma_start(
                                out=dbg_rec[2 * h + hf : 2 * h + hf + 1, :], in_=rec0[:]
                            )
                        bcast = pwb.tile([DH, 1024], F32, tag="bcast", name=f"bc{h}{hf}")
                        nc.gpsimd.partition_broadcast(bcast[:], rec0[:])
                        nc.vector.tensor_mul(
                            o_sb[po : po + 64, ch, jsl], ov[hf][0:DH, :], bcast[:]
                        )
                    if h == 3:
                        # WAW anchors: tiny writes into og gated on the h2/h3
                        # tail muls, so the scheduler cannot consider the og
                        # gathers "ready" before the tails and park their
                        # AG-completion waits ahead of the tail broadcasts in
                        # the gpsimd queue
                        for kc in range(KC):
                            nc.vector.tensor_copy(og[:, kc, 0:1], o_sb[:, 1, 0:1])
                        # ch0 gathers (AG0 completed during h2/h3, so these run
                        # instantly); phase C even-kc matmuls overlap AG1
                        for kc in range(0, KC, FC):
                            nc.gpsimd.indirect_dma_start(
                                out=og[:, kc, :],
                                out_offset=None,
                                in_=ag_out[0][:].rearrange("r (j n) -> (r j) n", n=512),
                                in_offset=bass.IndirectOffsetOnAxis(
                                    ap=gidx_sb[:, kc : kc + 1], axis=0
                                ),
                            )
                    if h % 2 == 1:
                        nc.sync.dma_start(out=ag_in[ch][:], in_=o_sb[:, ch, :])
                        nc.gpsimd.collective_compute(
                            "AllGather",
                            mybir.AluOpType.bypass,
                            replica_groups=GROUP8,
                            ins=[ag_in[ch][:]],
                            outs=[ag_out[ch][:]],
                        )

                # ch1 gathers wait on AG1 (the gpsimd queue is done with all
                # per-head work by now, so the wait blocks nothing)
                for kc in range(1, KC, FC):
                    nc.gpsimd.indirect_dma_start(
                        out=og[:, kc, :],
                        out_offset=None,
                        in_=ag_out[1][:].rearrange("r (j n) -> (r j) n", n=512),
                        in_offset=bass.IndirectOffsetOnAxis(
                            ap=gidx_sb[:, kc : kc + 1], axis=0
                        ),
                    )

            if dbg:
                nc.gpsimd.dma_start(out=dbg_qn[:], in_=qn[:])
                nc.gpsimd.dma_start(out=dbg_kt[:], in_=kT[:])
                nc.sync.dma_start(out=dbg_ri[:], in_=rinv[:])
                nc.sync.dma_start(out=dbg_rm[:], in_=rmsk_col[:])
                nc.sync.dma_start(out=dbg_va[:], in_=v_aug[:])
                nc.sync.dma_start(out=dbg_osb[:], in_=o_sb[:])
                nc.sync.dma_start(out=dbg_og[:], in_=og[:])

            # ========= PHASE C: output projection ==========================
            # even (ch 0) contractions first: they are ready while the ch 1
            # AllGather is still in flight
            kc_order = list(range(0, KC, 2)) + list(range(1, KC, 2))
            with (
                tc.tile_pool(name="pc", bufs=2) as pc,
                tc.tile_pool(name="psC", bufs=2, space="PSUM") as psC,
            ):
                for ntl in range(4):
                    yps = psC.tile([128, D], F32, tag="y", name=f"y{ntl}")
                    for dc in range(2):
                        dsl = slice(dc * 512, (dc + 1) * 512)
                        for i, kc in enumerate(kc_order):
                            nc.tensor.matmul(
                                yps[:, dsl],
                                og[:, kc, ntl * 128 : (ntl + 1) * 128],
                                wo_sb[:, kc, dsl],
                                start=(i == 0),
                                stop=False,
                            )
                        nc.tensor.matmul(
                            yps[:, dsl],
                            ones_row_bf[:],
                            bo_sb[:, dsl],
                            start=False,
                            stop=True,
                        )
                    ysb = pc.tile([128, D], F32, tag="ysb", name=f"ysb{ntl}")
                    nc.vector.tensor_copy(ysb[:], yps[:])
                    nc.sync.dma_start(
                        out=out_d[ntl * 128 : (ntl + 1) * 128, :], in_=ysb[:]
                    )

    nc.compile()
    return nc


def _rope_tables():
    """cos/sin tables matching the reference's f32 angle computation.

    C[d, n] = cos(n * theta[d//2]);  Ssw[2i] = +sin, Ssw[2i+1] = -sin
    (Ssw is the swapped-operand multiplier: rope = x*C + swap(x*Ssw)).
    Tiled x2 along partitions to cover a 2-head (128-row) chunk.
    """
    i2 = np.arange(0, DH, 2, dtype=np.float32)
    theta = (1.0 / (10000.0 ** (i2 / DH))).astype(np.float32)  # (32,)
    ang = np.arange(N, dtype=np.float32)[:, None] * theta[None, :]  # (N, 32) f32
    cos = np.cos(ang.astype(np.float64)).astype(np.float32).T  # (32, N)
    sin = np.sin(ang.astype(np.float64)).astype(np.float32).T
    cos_d = np.repeat(cos, 2, axis=0)  # (64, N)
    ssw = np.repeat(sin, 2, axis=0)
    ssw[1::2, :] *= -1.0
    cos_t = np.tile(cos_d, (2, 1)).astype(np.float32)  # (128, N)
    sin_t = np.tile(ssw, (2, 1)).astype(np.float32)
    return cos_t, sin_t


def _rearr(w):
    # [D, F] -> [128, KC, F] grouping the contraction dim into 128-row chunks
    d, f = w.shape
    return np.ascontiguousarray(
        w.reshape(KC, 128, f).transpose(1, 0, 2).astype(BF)
    )


def kernel(x, Wq, Wkv, norm_q_w, norm_k_w, Wo, bo, _trace=False, _dbg=False):
    global _CACHED_NC
    x = np.asarray(x, dtype=np.float32)
    Wq = np.asarray(Wq, dtype=np.float32)
    Wkv = np.asarray(Wkv, dtype=np.float32)
    norm_q_w = np.asarray(norm_q_w, dtype=np.float32)
    norm_k_w = np.asarray(norm_k_w, dtype=np.float32)
    Wo = np.asarray(Wo, dtype=np.float32)
    bo = np.asarray(bo, dtype=np.float32)

    cos_t, sin_t = _rope_tables()
    if _dbg:
        nc = build(dbg=True)
    else:
        if _CACHED_NC is None:
            _CACHED_NC = build()
        nc = _CACHED_NC

    in_maps = []
    for c in range(CORES):
        b, g = c // 4, c % 4
        fsl = slice(g * FPC, (g + 1) * FPC)
        gidx = np.empty((128, KC), dtype=np.int32)
        p = np.arange(128)
        for kc in range(KC):
            gidx[:, kc] = (128 * (4 * b + kc // FC) + p) * NJ + g
        in_maps.append(
            {
                "xT": _rearr(np.ascontiguousarray(x[b].T)),
                "wq": _rearr(Wq[:, fsl]),
                "wk": _rearr(Wkv[:, fsl]),
                "wv": _rearr(Wkv[:, D + g * FPC : D + (g + 1) * FPC]),
                "wo": _rearr(Wo),
                "bo": bo.reshape(1, D).astype(BF),
                "wqc": np.ascontiguousarray(
                    norm_q_w[fsl].reshape(FC, 128).T
                ).astype(BF),
                "wkc": np.ascontiguousarray(
                    norm_k_w[fsl].reshape(FC, 128).T
                ).astype(BF),
                "cos_t": cos_t.astype(BF),
                "sin_t": sin_t.astype(BF),
                "gidx": gidx,
            }
        )

    res = run_bass_kernel_spmd(nc, in_maps, list(range(CORES)), trace=_trace)
    out = np.empty((B, N, D), dtype=np.float32)
    for c in range(CORES):
        b, g = c // 4, c % 4
        out[b, g * 512 : (g + 1) * 512, :] = res.results[c]["out"]
    if _trace or _dbg:
        return out, res
    return out
